# revision 3
# baseline (speedup 1.0000x reference)
"""2-layer GAT (PyG-style) on TRN2, 8 NeuronCores.

Strategy: degree-bucketed node layout. Nodes sorted by in-degree (desc),
grouped into 128-node blocks; block b -> core b%8, tile t=b//8 (49 tiles/core).
Each tile is [128 node-partitions x d_t edge slots] with per-tile uniform
slot count (padding to sentinel). Per-edge source features are fetched with
dma_gather (custom GPSIMD bulk gather; signed int16 indices against a
mid-table base row 32768). The segment softmax + weighted aggregation are
pure row-wise DVE ops (multiply + pairwise-tree sum along the free dim).

Layer-1 table rows (bf16, 512B): [h (192, (c,h)-interleaved) | a_s 3xf32 |
a_d 3xf32 | pad]. Interleaving features as (c,h) keeps every operand of the
big per-edge multiply at innermost stride 1 => DVE 2x mode.
Layer-2 table rows (bf16, 256B): [h2 (64) | a_s2 f32 | a_d2 f32 | pad].

Softmax uses exp(leaky_relu(a_s+a_d)) without max-subtraction (logits are
O(10), no overflow in f32), normalized once per node: out = U / max(s,tiny).

Two NEFF launches: NEFF1 = dense1 (full, redundant per core) + L1 edge
phase + normalize/ELU + dense2 -> per-core table2 shard. Host assembles the
global table2. NEFF2 = L2 edge phase -> output shard. Host unpermutes.
"""
import numpy as np
import ml_dtypes

N = 50000
E = 800000
IN = 128
HID = 64
HEADS = 3
OUT = 64
NCORE = 8
P = 128
NBLK = (N + P - 1) // P            # 391
NBLKP = ((NBLK + NCORE - 1) // NCORE) * NCORE  # 392
T = NBLKP // NCORE                 # 49 tiles per core
G = 7                              # tiles per nid-gather group
NGRP = T // G                      # 7
SENT = N                           # sentinel table row
NROW = N + 1
BASE = 32768                       # gather base row (signed int16 trick)
E1 = 256                           # L1 row elems (bf16)
E2 = 128                           # L2 row elems (bf16)
NEG_SLOPE = 0.2
F1 = HEADS * HID                   # 192

bf16 = ml_dtypes.bfloat16

# interleaved feature order: row position i=(c*3+h) holds feature f=h*64+c
_POS = np.arange(F1)
PERM = (_POS % HEADS) * HID + (_POS // HEADS)   # row pos i -> original feature


def _pack_idx(rows_flat):
    """rows_flat int64[nidx] (table rows, nidx%128==0) -> int16 [128, nidx//16]
    wrap-16 layout replicated across the 8 groups of 16 partitions."""
    v = (rows_flat - BASE).astype(np.int16)
    cid = len(v) // 16
    block = v.reshape(cid, 16).T
    return np.tile(block, (8, 1))


def preprocess(edge_index):
    src = np.concatenate([edge_index[0].astype(np.int64), np.arange(N, dtype=np.int64)])
    dst = np.concatenate([edge_index[1].astype(np.int64), np.arange(N, dtype=np.int64)])
    deg = np.bincount(dst, minlength=N)
    order = np.argsort(-deg, kind="stable")          # node processing order
    eorder = np.argsort(dst, kind="stable")
    esrc = src[eorder]                               # edge srcs grouped by dst
    eptr = np.zeros(N + 1, np.int64)
    eptr[1:] = np.cumsum(deg)

    # per-tile slot counts (shared across cores - SPMD)
    d_t = []
    for t in range(T):
        g0 = (t * NCORE) * P
        dmax = int(deg[order[g0]]) if g0 < N else 1
        bump = 0
        for c in range(NCORE):
            gi = (t * NCORE + c) * P + (P - 1)
            if gi < N and int(deg[order[gi]]) >= dmax:
                bump = 1
        d_t.append(max(dmax + bump, 2))
    d_t = np.array(d_t, np.int64)

    # node grid per core: nid[c][t, p] = node id or SENT
    gidx = (np.arange(T)[:, None] * NCORE)[None] * P  # placeholder
    nid = np.full((NCORE, T, P), SENT, np.int64)
    for c in range(NCORE):
        gi = ((np.arange(T) * NCORE + c)[:, None]) * P + np.arange(P)[None, :]
        valid = gi < N
        nid[c][valid] = order[gi[valid]]

    # edge slot grids + packed indices
    idxe = np.zeros((NCORE, P, int(8 * d_t.sum())), np.int16)
    idxn = np.zeros((NCORE, P, NGRP * (G + 1) * 8), np.int16)
    for c in range(NCORE):
        eblocks = []
        for t in range(T):
            d = int(d_t[t])
            rows = np.full((P, d), SENT, np.int64)
            for p in range(P):
                n = nid[c, t, p]
                if n != SENT:
                    k = int(deg[n])
                    rows[p, :k] = esrc[eptr[n]:eptr[n] + k]
            eblocks.append(_pack_idx(rows.T.reshape(-1)))
        idxe[c] = np.concatenate(eblocks, axis=1)
        nblocks = []
        for g in range(NGRP):
            rows = np.full((P, G + 1), SENT, np.int64)
            for j in range(G):
                rows[:, j] = nid[c, g * G + j]
            nblocks.append(_pack_idx(rows.T.reshape(-1)))
        idxn[c] = np.concatenate(nblocks, axis=1)

    return dict(order=order, nid=nid, d_t=d_t, idxe=idxe, idxn=idxn,
                deg=deg, esrc=esrc, eptr=eptr)


def host_weights(x, W1, att_src1, att_dst1, b1, W2, att_src2, att_dst2, b2):
    W1s = np.stack([W1[:, h * HID:(h + 1) * HID] @ att_src1[h] for h in range(HEADS)], 1)  # [128,3]
    W1d = np.stack([W1[:, h * HID:(h + 1) * HID] @ att_dst1[h] for h in range(HEADS)], 1)
    W1e = np.concatenate([W1[:, PERM], W1s, W1d], axis=1)           # [128,198]
    W2e = np.concatenate([W2, (W2 @ att_src2[0])[:, None], (W2 @ att_dst2[0])[:, None]], 1)  # [192,66]
    W2e = W2e[PERM, :]                                               # rows in interleaved order
    xT = np.zeros((IN, NBLK * P), np.float32)
    xT[:, :N] = x.T
    sent1 = np.zeros(E1, bf16)
    sent1.view(np.float32)[96:99] = -1e30   # a_s = -inf-ish
    b1i = b1[PERM].astype(bf16)
    return dict(xT=xT.astype(bf16), W1e=W1e.astype(bf16),
                W2e1=W2e[:128].astype(bf16), W2e2=W2e[128:].astype(bf16),
                sent1=sent1.reshape(1, E1), b1i=np.tile(b1i, (P, 1)),
                b2b=np.tile(b2.astype(np.float32), (P, 1)))


# ---------------------------------------------------------------- emulation
def _bf(a):
    return a.astype(bf16).astype(np.float32)


def emulate(x, edge_index, W1, att_src1, att_dst1, b1, W2, att_src2, att_dst2, b2):
    """numpy emulation of the device pipeline (bf16 rounding where the HW has it)."""
    pre = preprocess(edge_index)
    hw = host_weights(x, W1, att_src1, att_dst1, b1, W2, att_src2, att_dst2, b2)
    d_t, nid = pre["d_t"], pre["nid"]
    deg, esrc, eptr = pre["deg"], pre["esrc"], pre["eptr"]

    # dense1 (as the device does: bf16 x, bf16 W1e, f32 accum)
    xTf = hw["xT"].astype(np.float32)[:, :N]
    W1ef = hw["W1e"].astype(np.float32)
    H = xTf.T @ W1ef                      # [N, 198] f32
    tab_h = _bf(H[:, :F1])                # bf16 h (interleaved)
    tab_as = H[:, 192:195]
    tab_ad = H[:, 195:198]
    # sentinel
    tab_h = np.vstack([tab_h, np.zeros((1, F1), np.float32)])
    tab_as = np.vstack([tab_as, np.full((1, HEADS), -1e30, np.float32)])
    tab_ad = np.vstack([tab_ad, np.zeros((1, HEADS), np.float32)])

    tab2_h = np.zeros((NROW, OUT), np.float32)
    tab2_as = np.zeros((NROW, 1), np.float32)
    tab2_ad = np.zeros((NROW, 1), np.float32)
    tab2_as[SENT] = -1e30
    W2e1f = hw["W2e1"].astype(np.float32)
    W2e2f = hw["W2e2"].astype(np.float32)
    b1f = hw["b1i"].astype(np.float32)[0]

    for c in range(NCORE):
        for t in range(T):
            d = int(d_t[t])
            rows = np.full((P, d), SENT, np.int64)
            for p in range(P):
                n = nid[c, t, p]
                if n != SENT:
                    k = int(deg[n])
                    rows[p, :k] = esrc[eptr[n]:eptr[n] + k]
            g_h = tab_h[rows]                       # [P,d,192] bf16-valued
            g_as = tab_as[rows]                     # [P,d,3] f32
            a_d = tab_ad[nid[c, t]]                 # [P,3]
            e = g_as + a_d[:, None, :]
            e = np.maximum(e, NEG_SLOPE * e)
            w = np.exp(e)                           # f32
            wb = _bf(w)
            s = w.sum(axis=1)                       # [P,3] f32 (reduce)
            prod = _bf(g_h * wb.repeat(HID, axis=2).reshape(P, d, HEADS, HID).transpose(0, 1, 3, 2).reshape(P, d, F1))
            # ^ interleaved: row pos i=(c*3+h) -> multiply by w[...,h]
            # tree-sum over d in bf16
            vals = [prod[:, j] for j in range(d)]
            while len(vals) > 1:
                nxt = []
                for j in range(0, len(vals) - 1, 2):
                    nxt.append(_bf(vals[j] + vals[j + 1]))
                if len(vals) % 2:
                    nxt.append(vals[-1])
                vals = nxt
            U = vals[0]                             # [P,192] bf16-valued
            r = _bf(1.0 / np.maximum(s, 1e-30))     # [P,3]
            h1 = _bf(U * np.repeat(r[:, None, :], HID, 1).reshape(P, F1))
            h1 = _bf(h1 + b1f)
            h1 = _bf(np.maximum(h1, 0) + _bf(np.exp(np.minimum(h1, 0))) - 1)
            # dense2
            out2 = _bf(h1) @ np.concatenate([W2e1f, W2e2f], 0)   # [P,66] f32 accum
            valid = nid[c, t] != SENT
            nn = nid[c, t][valid]
            tab2_h[nn] = _bf(out2[valid, :OUT])
            tab2_as[nn, 0] = out2[valid, OUT]
            tab2_ad[nn, 0] = out2[valid, OUT + 1]

    outp = np.zeros((N, OUT), np.float32)
    b2f = hw["b2b"][0]
    for c in range(NCORE):
        for t in range(T):
            d = int(d_t[t])
            rows = np.full((P, d), SENT, np.int64)
            for p in range(P):
                n = nid[c, t, p]
                if n != SENT:
                    k = int(deg[n])
                    rows[p, :k] = esrc[eptr[n]:eptr[n] + k]
            g_h = tab2_h[rows]                      # [P,d,64]
            g_as = tab2_as[rows]                    # [P,d,1]
            a_d = tab2_ad[nid[c, t]]                # [P,1]
            e = g_as + a_d[:, None, :]
            e = np.maximum(e, NEG_SLOPE * e)
            w = np.exp(e)
            wb = _bf(w)
            s = w.sum(axis=1)                       # [P,1]
            prod = _bf(g_h * wb)
            vals = [prod[:, j] for j in range(d)]
            while len(vals) > 1:
                nxt = []
                for j in range(0, len(vals) - 1, 2):
                    nxt.append(_bf(vals[j] + vals[j + 1]))
                if len(vals) % 2:
                    nxt.append(vals[-1])
                vals = nxt
            U2 = vals[0]
            r = 1.0 / np.maximum(s, 1e-30)
            o = U2 * r[:, None].reshape(P, 1) + b2f
            valid = nid[c, t] != SENT
            outp[nid[c, t][valid]] = o[valid]
    return outp


# ---------------------------------------------------------------- bass build
def _build_neff1(d_t, mul_gp_mod=3):
    import concourse.bass as bass
    import concourse.bacc as bacc
    import concourse.mybir as mybir
    import concourse.tile as tile
    from concourse.masks import make_identity

    dt = mybir.dt
    nc = bacc.Bacc(num_swdge_queues=4)
    CE = int(8 * int(np.sum(d_t)))
    CN = NGRP * (G + 1) * 8
    xT = nc.dram_tensor("xT", [IN, NBLK * P], dt.bfloat16, kind="ExternalInput")
    W1e = nc.dram_tensor("W1e", [IN, 198], dt.bfloat16, kind="ExternalInput")
    W2e1 = nc.dram_tensor("W2e1", [128, 66], dt.bfloat16, kind="ExternalInput")
    W2e2 = nc.dram_tensor("W2e2", [64, 66], dt.bfloat16, kind="ExternalInput")
    b1i = nc.dram_tensor("b1i", [P, F1], dt.bfloat16, kind="ExternalInput")
    sent1 = nc.dram_tensor("sent1", [1, E1], dt.bfloat16, kind="ExternalInput")
    idxe = nc.dram_tensor("idxe", [P, CE], dt.int16, kind="ExternalInput")
    idxn = nc.dram_tensor("idxn", [P, CN], dt.int16, kind="ExternalInput")
    tab2loc = nc.dram_tensor("tab2loc", [T * P, 68], dt.bfloat16, kind="ExternalOutput")
    tab1 = nc.dram_tensor("tab1", [NROW, E1], dt.bfloat16)

    FQ = [0]

    def q():
        FQ[0] = (FQ[0] + 1) % 4
        return FQ[0]

    with tile.TileContext(nc) as tc:
        with tc.tile_pool(name="const", bufs=1) as cp, \
             tc.tile_pool(name="xp", bufs=3) as xp, \
             tc.tile_pool(name="rowp", bufs=3) as rowp, \
             tc.tile_pool(name="gp", bufs=2) as gpool, \
             tc.tile_pool(name="np_", bufs=2) as npool, \
             tc.tile_pool(name="wp", bufs=2) as wpool, \
             tc.tile_pool(name="trp", bufs=2) as trp, \
             tc.tile_pool(name="hp", bufs=2) as hpool, \
             tc.tile_pool(name="psA", bufs=2, space="PSUM") as psA, \
             tc.tile_pool(name="psB", bufs=2, space="PSUM") as psB:

            w1_sb = cp.tile([IN, 198], dt.bfloat16)
            nc.sync.dma_start(out=w1_sb[:], in_=W1e[:, :])
            w2a_sb = cp.tile([128, 66], dt.bfloat16)
            nc.sync.dma_start(out=w2a_sb[:], in_=W2e1[:, :])
            w2b_sb = cp.tile([64, 66], dt.bfloat16)
            nc.sync.dma_start(out=w2b_sb[:], in_=W2e2[:, :])
            b1_sb = cp.tile([P, F1], dt.bfloat16)
            nc.sync.dma_start(out=b1_sb[:], in_=b1i[:, :])
            ide = cp.tile([P, P], dt.bfloat16)
            make_identity(nc, ide[:])
            ie_sb = cp.tile([P, CE], dt.int16)
            nc.sync.dma_start(out=ie_sb[:], in_=idxe[:, :])
            in_sb = cp.tile([P, CN], dt.int16)
            nc.sync.dma_start(out=in_sb[:], in_=idxn[:, :])
            sent_sb = cp.tile([1, E1], dt.bfloat16)
            nc.sync.dma_start(out=sent_sb[:], in_=sent1[:, :])
            nc.sync.dma_start(out=tab1[SENT:SENT + 1, :], in_=sent_sb[:])

            # ---------------- dense1: tab1 rows for all nodes
            XC = 4                        # x tiles per DMA chunk
            for ch in range((NBLK + XC - 1) // XC):
                ntile = min(XC, NBLK - ch * XC)
                xch = xp.tile([P, XC * P], dt.bfloat16, tag="x")
                nc.sync.dma_start(
                    out=xch[:, :ntile * P],
                    in_=xT[:, ch * XC * P: ch * XC * P + ntile * P])
                for k in range(ntile):
                    nt = ch * XC + k
                    pt = psA.tile([P, 198], dt.float32, tag="d1")
                    nc.tensor.matmul(pt[:], lhsT=xch[:, k * P:(k + 1) * P],
                                     rhs=w1_sb[:], start=True, stop=True)
                    rt = rowp.tile([P, E1], dt.bfloat16, tag="rt")
                    nc.scalar.activation(rt[:, :F1], pt[:, :F1],
                                         mybir.ActivationFunctionType.Copy)
                    nc.vector.tensor_copy(out=rt[:, F1:F1 + 12].bitcast(dt.float32),
                                          in_=pt[:, F1:198])
                    nrows = min(P, N - nt * P)
                    nc.sync.dma_start(out=tab1[nt * P: nt * P + nrows, :],
                                      in_=rt[:nrows, :])

            # ---------------- edge phase L1 + dense2
            tab_lo = tab1[BASE:, :]
            off_e = 0
            for g in range(NGRP):
                ndt = npool.tile([P, (G + 1) * E1], dt.bfloat16, tag="nd")
                nc.gpsimd.dma_gather(
                    out_ap=ndt[:].rearrange("p (j e) -> p j e", e=E1),
                    in_ap=tab_lo,
                    idxs_ap=in_sb[:, g * (G + 1) * 8:(g + 1) * (G + 1) * 8],
                    num_idxs=(G + 1) * P, num_idxs_reg=(G + 1) * P,
                    elem_size=E1, single_packet=False, queue_num=q())
                ndf = ndt[:].bitcast(dt.float32)       # [P, (G+1)*128]
                for j in range(G):
                    t = g * G + j
                    d = int(d_t[t])
                    pt = gpool.tile([P, d * E1], dt.bfloat16, tag="pay")
                    nc.gpsimd.dma_gather(
                        out_ap=pt[:].rearrange("p (s e) -> p s e", e=E1),
                        in_ap=tab_lo,
                        idxs_ap=ie_sb[:, off_e: off_e + 8 * d],
                        num_idxs=d * P, num_idxs_reg=d * P,
                        elem_size=E1, single_packet=False, queue_num=q())
                    off_e += 8 * d
                    ptf = pt[:].bitcast(dt.float32)    # [P, d*128] f32 view
                    a_s = ptf.rearrange("p (s e) -> p s e", e=128)[:, :, 96:99]
                    a_d = ndf.rearrange("p (j e) -> p j e", e=128)[:, j, 99:102]

                    et = wpool.tile([P, d * 3], dt.float32, tag="e")
                    nc.vector.tensor_tensor(
                        out=et[:].rearrange("p (s h) -> p s h", h=3),
                        in0=a_s, in1=a_d.unsqueeze(1).to_broadcast([P, d, 3]),
                        op=mybir.AluOpType.add)
                    et2 = wpool.tile([P, d * 3], dt.float32, tag="e2")
                    nc.vector.scalar_tensor_tensor(
                        out=et2[:], in0=et[:], scalar=NEG_SLOPE, in1=et[:],
                        op0=mybir.AluOpType.mult, op1=mybir.AluOpType.max)
                    wf = wpool.tile([P, d * 3], dt.float32, tag="wf")
                    nc.scalar.activation(wf[:], et2[:], mybir.ActivationFunctionType.Exp)
                    wb = wpool.tile([P, d * 3], dt.bfloat16, tag="wb")
                    nc.vector.tensor_copy(out=wb[:], in_=wf[:])
                    st = wpool.tile([P, 3], dt.float32, tag="s")
                    nc.vector.tensor_reduce(
                        out=st[:],
                        in_=wf[:].rearrange("p (s h) -> p h s", h=3),
                        axis=mybir.AxisListType.X, op=mybir.AluOpType.add)
                    # prod = h * w  (all operands innermost step-1: 2x mode)
                    pr = trp.tile([P, d * F1], dt.bfloat16, tag="pr0")
                    h_view = pt[:].rearrange("p (s e) -> p s e", e=E1)[:, :, :F1] \
                                  .rearrange("p s (c h) -> p s c h", h=3)
                    w_view = wb[:].rearrange("p (s h) -> p s h", h=3) \
                                  .unsqueeze(2).to_broadcast([P, d, HID, 3])
                    eng = nc.gpsimd if (t % mul_gp_mod == mul_gp_mod - 1) else nc.vector
                    eng.tensor_tensor(
                        out=pr[:].rearrange("p (s c h) -> p s c h", h=3, c=HID),
                        in0=h_view, in1=w_view, op=mybir.AluOpType.mult)
                    # pairwise tree over slots
                    cur, n = pr, d
                    lvl = 0
                    while n > 1:
                        half, odd = n // 2, n % 2
                        dst_t = trp.tile([P, (half + odd) * F1], dt.bfloat16,
                                         tag=f"tr{1 + (lvl % 2)}")
                        nc.vector.tensor_tensor(
                            out=dst_t[:].rearrange("p (s f) -> p s f", f=F1)[:, :half],
                            in0=cur[:, :n * F1].rearrange("p (s f) -> p s f", f=F1)[:, 0:2 * half:2],
                            in1=cur[:, :n * F1].rearrange("p (s f) -> p s f", f=F1)[:, 1:2 * half:2],
                            op=mybir.AluOpType.add)
                        if odd:
                            nc.vector.tensor_copy(
                                out=dst_t[:, half * F1:(half + 1) * F1],
                                in_=cur[:, (n - 1) * F1:n * F1])
                        cur, n = dst_t, half + odd
                        lvl += 1
                    # normalize + bias + ELU (bf16)
                    sc = wpool.tile([P, 3], dt.float32, tag="sc")
                    nc.vector.tensor_scalar_max(out=sc[:], in0=st[:], scalar1=1e-30)
                    rc = wpool.tile([P, 3], dt.float32, tag="rc")
                    nc.vector.reciprocal(out=rc[:], in_=sc[:])
                    rcb = wpool.tile([P, 3], dt.bfloat16, tag="rcb")
                    nc.vector.tensor_copy(out=rcb[:], in_=rc[:])
                    h1 = hpool.tile([P, F1], dt.bfloat16, tag="h1")
                    nc.vector.tensor_tensor(
                        out=h1[:].rearrange("p (c h) -> p c h", h=3),
                        in0=cur[:, :F1].rearrange("p (c h) -> p c h", h=3),
                        in1=rcb[:].unsqueeze(1).to_broadcast([P, HID, 3]),
                        op=mybir.AluOpType.mult)
                    nc.vector.tensor_add(out=h1[:], in0=h1[:], in1=b1_sb[:])
                    tmin = hpool.tile([P, F1], dt.bfloat16, tag="tmin")
                    nc.vector.tensor_scalar_min(out=tmin[:], in0=h1[:], scalar1=0.0)
                    texp = hpool.tile([P, F1], dt.bfloat16, tag="texp")
                    nc.scalar.activation(texp[:], tmin[:], mybir.ActivationFunctionType.Exp)
                    h1e = hpool.tile([P, F1], dt.bfloat16, tag="h1e")
                    nc.vector.scalar_tensor_tensor(
                        out=h1e[:], in0=h1[:], scalar=0.0, in1=texp[:],
                        op0=mybir.AluOpType.max, op1=mybir.AluOpType.add)
                    nc.vector.tensor_scalar_add(out=h1e[:], in0=h1e[:], scalar1=-1.0)
                    # transpose 192 = 128 + 64
                    tp1 = psB.tile([P, P], dt.bfloat16, tag="tp1")
                    nc.tensor.transpose(tp1[:], h1e[:, :P], ide[:])
                    tp2 = psB.tile([64, P], dt.bfloat16, tag="tp2")
                    nc.tensor.transpose(tp2[:], h1e[:, P:F1], ide[:])
                    hT1 = hpool.tile([P, P], dt.bfloat16, tag="hT1")
                    nc.vector.tensor_copy(out=hT1[:], in_=tp1[:])
                    hT2 = hpool.tile([64, P], dt.bfloat16, tag="hT2")
                    nc.vector.tensor_copy(out=hT2[:], in_=tp2[:])
                    o2 = psA.tile([P, 66], dt.float32, tag="o2")
                    nc.tensor.matmul(o2[:], lhsT=hT1[:], rhs=w2a_sb[:], start=True, stop=False)
                    nc.tensor.matmul(o2[:], lhsT=hT2[:], rhs=w2b_sb[:], start=False, stop=True)
                    r2 = rowp.tile([P, 68], dt.bfloat16, tag="r2")
                    nc.scalar.activation(r2[:, :OUT], o2[:, :OUT],
                                         mybir.ActivationFunctionType.Copy)
                    nc.vector.tensor_copy(out=r2[:, OUT:OUT + 4].bitcast(dt.float32),
                                          in_=o2[:, OUT:OUT + 2])
                    nc.sync.dma_start(out=tab2loc[t * P:(t + 1) * P, :], in_=r2[:])
    nc.compile()
    return nc


def _build_neff2(d_t, mul_gp_mod=3):
    import concourse.bass as bass
    import concourse.bacc as bacc
    import concourse.mybir as mybir
    import concourse.tile as tile

    dt = mybir.dt
    nc = bacc.Bacc(num_swdge_queues=4)
    CE = int(8 * int(np.sum(d_t)))
    CN = NGRP * (G + 1) * 8
    tab2 = nc.dram_tensor("tab2", [NROW, E2], dt.bfloat16, kind="ExternalInput")
    idxe = nc.dram_tensor("idxe", [P, CE], dt.int16, kind="ExternalInput")
    idxn = nc.dram_tensor("idxn", [P, CN], dt.int16, kind="ExternalInput")
    b2b = nc.dram_tensor("b2b", [P, OUT], dt.float32, kind="ExternalInput")
    out2 = nc.dram_tensor("out2", [T * P, OUT], dt.float32, kind="ExternalOutput")

    FQ = [0]

    def q():
        FQ[0] = (FQ[0] + 1) % 4
        return FQ[0]

    with tile.TileContext(nc) as tc:
        with tc.tile_pool(name="const", bufs=1) as cp, \
             tc.tile_pool(name="gp", bufs=2) as gpool, \
             tc.tile_pool(name="np_", bufs=2) as npool, \
             tc.tile_pool(name="wp", bufs=2) as wpool, \
             tc.tile_pool(name="trp", bufs=2) as trp, \
             tc.tile_pool(name="op", bufs=3) as opool:
            ie_sb = cp.tile([P, CE], dt.int16)
            nc.sync.dma_start(out=ie_sb[:], in_=idxe[:, :])
            in_sb = cp.tile([P, CN], dt.int16)
            nc.sync.dma_start(out=in_sb[:], in_=idxn[:, :])
            b2_sb = cp.tile([P, OUT], dt.float32)
            nc.sync.dma_start(out=b2_sb[:], in_=b2b[:, :])

            tab_lo = tab2[BASE:, :]
            off_e = 0
            for g in range(NGRP):
                ndt = npool.tile([P, (G + 1) * E2], dt.bfloat16, tag="nd")
                nc.gpsimd.dma_gather(
                    out_ap=ndt[:].rearrange("p (j e) -> p j e", e=E2),
                    in_ap=tab_lo,
                    idxs_ap=in_sb[:, g * (G + 1) * 8:(g + 1) * (G + 1) * 8],
                    num_idxs=(G + 1) * P, num_idxs_reg=(G + 1) * P,
                    elem_size=E2, single_packet=False, queue_num=q())
                ndf = ndt[:].bitcast(dt.float32)     # [P, (G+1)*64]
                for j in range(G):
                    t = g * G + j
                    d = int(d_t[t])
                    pt = gpool.tile([P, d * E2], dt.bfloat16, tag="pay")
                    nc.gpsimd.dma_gather(
                        out_ap=pt[:].rearrange("p (s e) -> p s e", e=E2),
                        in_ap=tab_lo,
                        idxs_ap=ie_sb[:, off_e: off_e + 8 * d],
                        num_idxs=d * P, num_idxs_reg=d * P,
                        elem_size=E2, single_packet=False, queue_num=q())
                    off_e += 8 * d
                    ptf = pt[:].bitcast(dt.float32)   # [P, d*64]
                    a_s = ptf.rearrange("p (s e) -> p s e", e=64)[:, :, 32:33]
                    a_d = ndf.rearrange("p (j e) -> p j e", e=64)[:, j, 33:34]
                    et = wpool.tile([P, d], dt.float32, tag="e")
                    nc.vector.tensor_tensor(
                        out=et[:].unsqueeze(2),
                        in0=a_s, in1=a_d.unsqueeze(1).to_broadcast([P, d, 1]),
                        op=mybir.AluOpType.add)
                    et2 = wpool.tile([P, d], dt.float32, tag="e2")
                    nc.vector.scalar_tensor_tensor(
                        out=et2[:], in0=et[:], scalar=NEG_SLOPE, in1=et[:],
                        op0=mybir.AluOpType.mult, op1=mybir.AluOpType.max)
                    wf = wpool.tile([P, d], dt.float32, tag="wf")
                    nc.scalar.activation(wf[:], et2[:], mybir.ActivationFunctionType.Exp)
                    wb = wpool.tile([P, d], dt.bfloat16, tag="wb")
                    nc.vector.tensor_copy(out=wb[:], in_=wf[:])
                    st = wpool.tile([P, 1], dt.float32, tag="s")
                    nc.vector.tensor_reduce(out=st[:], in_=wf[:],
                                            axis=mybir.AxisListType.X,
                                            op=mybir.AluOpType.add)
                    pr = trp.tile([P, d * OUT], dt.bfloat16, tag="pr0")
                    h_view = pt[:].rearrange("p (s e) -> p s e", e=E2)[:, :, :OUT]
                    w_view = wb[:].unsqueeze(2).to_broadcast([P, d, OUT])
                    eng = nc.gpsimd if (t % mul_gp_mod == mul_gp_mod - 1) else nc.vector
                    eng.tensor_tensor(
                        out=pr[:].rearrange("p (s f) -> p s f", f=OUT),
                        in0=h_view, in1=w_view, op=mybir.AluOpType.mult)
                    cur, n = pr, d
                    lvl = 0
                    while n > 1:
                        half, odd = n // 2, n % 2
                        dst_t = trp.tile([P, (half + odd) * OUT], dt.bfloat16,
                                         tag=f"tr{1 + (lvl % 2)}")
                        nc.vector.tensor_tensor(
                            out=dst_t[:].rearrange("p (s f) -> p s f", f=OUT)[:, :half],
                            in0=cur[:, :n * OUT].rearrange("p (s f) -> p s f", f=OUT)[:, 0:2 * half:2],
                            in1=cur[:, :n * OUT].rearrange("p (s f) -> p s f", f=OUT)[:, 1:2 * half:2],
                            op=mybir.AluOpType.add)
                        if odd:
                            nc.vector.tensor_copy(
                                out=dst_t[:, half * OUT:(half + 1) * OUT],
                                in_=cur[:, (n - 1) * OUT:n * OUT])
                        cur, n = dst_t, half + odd
                        lvl += 1
                    sc = wpool.tile([P, 1], dt.float32, tag="sc")
                    nc.vector.tensor_scalar_max(out=sc[:], in0=st[:], scalar1=1e-30)
                    rc = wpool.tile([P, 1], dt.float32, tag="rc")
                    nc.vector.reciprocal(out=rc[:], in_=sc[:])
                    ot = opool.tile([P, OUT], dt.float32, tag="ot")
                    nc.vector.tensor_scalar(
                        out=ot[:], in0=cur[:, :OUT], scalar1=rc[:, :1], scalar2=None,
                        op0=mybir.AluOpType.mult)
                    nc.vector.tensor_add(out=ot[:], in0=ot[:], in1=b2_sb[:])
                    nc.sync.dma_start(out=out2[t * P:(t + 1) * P, :], in_=ot[:])
    nc.compile()
    return nc


# ---------------------------------------------------------------- kernel
def kernel(x, edge_index, W1, att_src1, att_dst1, b1, W2, att_src2, att_dst2, b2,
           _emulate=False, _timing=None):
    x = np.asarray(x, np.float32)
    edge_index = np.asarray(edge_index)
    W1 = np.asarray(W1, np.float32)
    att_src1 = np.asarray(att_src1, np.float32)
    att_dst1 = np.asarray(att_dst1, np.float32)
    b1 = np.asarray(b1, np.float32)
    W2 = np.asarray(W2, np.float32)
    att_src2 = np.asarray(att_src2, np.float32)
    att_dst2 = np.asarray(att_dst2, np.float32)
    b2 = np.asarray(b2, np.float32)

    if _emulate:
        return emulate(x, edge_index, W1, att_src1, att_dst1, b1,
                       W2, att_src2, att_dst2, b2)

    from concourse.bass_utils import run_bass_kernel_spmd

    pre = preprocess(edge_index)
    hw = host_weights(x, W1, att_src1, att_dst1, b1, W2, att_src2, att_dst2, b2)
    d_t, nid = pre["d_t"], pre["nid"]

    trace = _timing is not None

    # ---- NEFF1
    nc1 = _build_neff1(d_t)
    maps1 = [dict(xT=hw["xT"], W1e=hw["W1e"], W2e1=hw["W2e1"], W2e2=hw["W2e2"],
                  b1i=hw["b1i"], sent1=hw["sent1"],
                  idxe=pre["idxe"][c], idxn=pre["idxn"][c]) for c in range(NCORE)]
    res1 = run_bass_kernel_spmd(nc1, maps1, core_ids=list(range(NCORE)), trace=trace)

    # host: assemble global table2
    tab2 = np.zeros((NROW, E2), bf16)
    tab2[SENT].view(np.float32)[32] = -1e30
    for c in range(NCORE):
        loc = res1.results[c]["tab2loc"]           # [T*P, 68] bf16
        nn = nid[c].reshape(-1)                    # [T*P]
        valid = nn != SENT
        tab2[nn[valid], :68] = loc[valid]

    # ---- NEFF2
    nc2 = _build_neff2(d_t)
    maps2 = [dict(tab2=tab2, idxe=pre["idxe"][c], idxn=pre["idxn"][c],
                  b2b=hw["b2b"]) for c in range(NCORE)]
    res2 = run_bass_kernel_spmd(nc2, maps2, core_ids=list(range(NCORE)), trace=trace)

    out = np.zeros((N, OUT), np.float32)
    for c in range(NCORE):
        o = res2.results[c]["out2"]
        nn = nid[c].reshape(-1)
        valid = nn != SENT
        out[nn[valid]] = o[valid]

    if _timing is not None:
        _timing["neff1_ns"] = res1.exec_time_ns
        _timing["neff2_ns"] = res2.exec_time_ns
    return out


# revision 7
# speedup vs baseline: 1.5425x; 1.5425x over previous
"""2-layer GAT (PyG-style) on TRN2, 8 NeuronCores.

Strategy: degree-bucketed node layout. Nodes sorted by in-degree (desc),
grouped into 128-node blocks; block b -> core b%8, tile t=b//8 (49 tiles/core).
Each tile is [128 node-partitions x d_t edge slots] with per-tile uniform
slot count (padding to sentinel). Per-edge source features are fetched with
dma_gather (custom GPSIMD bulk gather; signed int16 indices against a
mid-table base row 32768). The segment softmax + weighted aggregation are
pure row-wise DVE ops (multiply + pairwise-tree sum along the free dim).

Layer-1 table rows (bf16, 512B): [h (192, (c,h)-interleaved) | a_s 3xf32 |
a_d 3xf32 | pad]. Interleaving features as (c,h) keeps every operand of the
big per-edge multiply at innermost stride 1 => DVE 2x mode.
Layer-2 table rows (bf16, 256B): [h2 (64) | a_s2 f32 | a_d2 f32 | pad].

Softmax uses exp(leaky_relu(a_s+a_d)) without max-subtraction (logits are
O(10), no overflow in f32), normalized once per node: out = U / max(s,tiny).

Two NEFF launches: NEFF1 = dense1 (full, redundant per core) + L1 edge
phase + normalize/ELU + dense2 -> per-core table2 shard. Host assembles the
global table2. NEFF2 = L2 edge phase -> output shard. Host unpermutes.
"""
import numpy as np
import ml_dtypes

N = 50000
E = 800000
IN = 128
HID = 64
HEADS = 3
OUT = 64
NCORE = 8
P = 128
NBLK = (N + P - 1) // P            # 391
NBLKP = ((NBLK + NCORE - 1) // NCORE) * NCORE  # 392
T = NBLKP // NCORE                 # 49 tiles per core
G = 7                              # tiles per nid-gather group
NGRP = T // G                      # 7
SENT = N                           # sentinel table row
NROW = N + 1
BASE = 32768                       # gather base row (signed int16 trick)
E1 = 256                           # L1 row elems (bf16)
E2 = 128                           # L2 row elems (bf16)
NEG_SLOPE = 0.2
F1 = HEADS * HID                   # 192
CHUNK = 8                          # max slots per dma_gather call

bf16 = ml_dtypes.bfloat16

# interleaved feature order: row position i=(c*3+h) holds feature f=h*64+c
_POS = np.arange(F1)
PERM = (_POS % HEADS) * HID + (_POS // HEADS)   # row pos i -> original feature


def _pack_idx(rows_flat):
    """rows_flat int64[nidx] (table rows, nidx%128==0) -> int16 [128, nidx//16]
    wrap-16 layout replicated across the 8 groups of 16 partitions."""
    v = (rows_flat - BASE).astype(np.int16)
    cid = len(v) // 16
    block = v.reshape(cid, 16).T
    return np.tile(block, (8, 1))


def preprocess(edge_index):
    src = np.concatenate([edge_index[0].astype(np.int64), np.arange(N, dtype=np.int64)])
    dst = np.concatenate([edge_index[1].astype(np.int64), np.arange(N, dtype=np.int64)])
    deg = np.bincount(dst, minlength=N)
    order = np.argsort(-deg, kind="stable")          # node processing order
    eorder = np.argsort(dst, kind="stable")
    esrc = src[eorder]                               # edge srcs grouped by dst
    eptr = np.zeros(N + 1, np.int64)
    eptr[1:] = np.cumsum(deg)

    # per-tile slot counts (shared across cores - SPMD).
    # Partition 127 needs its chunk-boundary-last slots to be sentinel pads
    # (the gather drops trailing-negative indices per call), so ensure
    # deg(p127) <= d - n_chunks.
    d_t = []
    for t in range(T):
        g0 = (t * NCORE) * P
        dmax = int(deg[order[g0]]) if g0 < N else 2
        d127 = 0
        for c in range(NCORE):
            gi = (t * NCORE + c) * P + (P - 1)
            if gi < N:
                d127 = max(d127, int(deg[order[gi]]))
        d = max(dmax, 2)
        while True:
            nch = (d + CHUNK - 1) // CHUNK
            need = max(dmax, d127 + nch, 2)
            if need <= d:
                break
            d = need
        d_t.append(d)
    d_t = np.array(d_t, np.int64)

    # node grid per core: nid[c][t, p] = node id or SENT
    gidx = (np.arange(T)[:, None] * NCORE)[None] * P  # placeholder
    nid = np.full((NCORE, T, P), SENT, np.int64)
    for c in range(NCORE):
        gi = ((np.arange(T) * NCORE + c)[:, None]) * P + np.arange(P)[None, :]
        valid = gi < N
        nid[c][valid] = order[gi[valid]]

    # edge slot grids + packed indices
    idxe = np.zeros((NCORE, P, int(8 * d_t.sum())), np.int16)
    idxn = np.zeros((NCORE, P, NGRP * (G + 1) * 8), np.int16)
    for c in range(NCORE):
        eblocks = []
        for t in range(T):
            d = int(d_t[t])
            rows = np.full((P, d), SENT, np.int64)
            nch = (d + CHUNK - 1) // CHUNK
            per = (d + nch - 1) // nch
            bnd = set()
            s0 = 0
            while s0 < d:
                s1 = min(s0 + per, d)
                bnd.add(s1 - 1)
                s0 = s1
            p127_slots = [s for s in range(d) if s not in bnd]
            for p in range(P):
                n = nid[c, t, p]
                if n != SENT:
                    k = int(deg[n])
                    if p == P - 1:
                        assert k <= len(p127_slots)
                        rows[p, p127_slots[:k]] = esrc[eptr[n]:eptr[n] + k]
                    else:
                        rows[p, :k] = esrc[eptr[n]:eptr[n] + k]
            eblocks.append(_pack_idx(rows.T.reshape(-1)))
        idxe[c] = np.concatenate(eblocks, axis=1)
        nblocks = []
        for g in range(NGRP):
            rows = np.full((P, G + 1), SENT, np.int64)
            for j in range(G):
                rows[:, j] = nid[c, g * G + j]
            nblocks.append(_pack_idx(rows.T.reshape(-1)))
        idxn[c] = np.concatenate(nblocks, axis=1)

    return dict(order=order, nid=nid, d_t=d_t, idxe=idxe, idxn=idxn,
                deg=deg, esrc=esrc, eptr=eptr)


def host_weights(x, W1, att_src1, att_dst1, b1, W2, att_src2, att_dst2, b2):
    W1s = np.stack([W1[:, h * HID:(h + 1) * HID] @ att_src1[h] for h in range(HEADS)], 1)  # [128,3]
    W1d = np.stack([W1[:, h * HID:(h + 1) * HID] @ att_dst1[h] for h in range(HEADS)], 1)
    W1e = np.concatenate([W1[:, PERM], W1s, W1d], axis=1)           # [128,198]
    W2e = np.concatenate([W2, (W2 @ att_src2[0])[:, None], (W2 @ att_dst2[0])[:, None]], 1)  # [192,66]
    W2e = W2e[PERM, :]                                               # rows in interleaved order
    xT = np.zeros((IN, NBLK * P), np.float32)
    xT[:, :N] = x.T
    sent1 = np.zeros(E1, bf16)
    sent1.view(np.float32)[96:99] = -1e30   # a_s = -inf-ish
    b1i = b1[PERM].astype(bf16)
    return dict(xT=xT.astype(bf16), W1e=W1e.astype(bf16),
                W2e1=W2e[:128].astype(bf16), W2e2=W2e[128:].astype(bf16),
                sent1=sent1.reshape(1, E1), b1i=np.tile(b1i, (P, 1)),
                b2b=np.tile(b2.astype(np.float32), (P, 1)))


# ---------------------------------------------------------------- emulation
def _bf(a):
    return a.astype(bf16).astype(np.float32)


def emulate(x, edge_index, W1, att_src1, att_dst1, b1, W2, att_src2, att_dst2, b2):
    """numpy emulation of the device pipeline (bf16 rounding where the HW has it)."""
    pre = preprocess(edge_index)
    hw = host_weights(x, W1, att_src1, att_dst1, b1, W2, att_src2, att_dst2, b2)
    d_t, nid = pre["d_t"], pre["nid"]
    deg, esrc, eptr = pre["deg"], pre["esrc"], pre["eptr"]

    # dense1 (as the device does: bf16 x, bf16 W1e, f32 accum)
    xTf = hw["xT"].astype(np.float32)[:, :N]
    W1ef = hw["W1e"].astype(np.float32)
    H = xTf.T @ W1ef                      # [N, 198] f32
    tab_h = _bf(H[:, :F1])                # bf16 h (interleaved)
    tab_as = H[:, 192:195]
    tab_ad = H[:, 195:198]
    # sentinel
    tab_h = np.vstack([tab_h, np.zeros((1, F1), np.float32)])
    tab_as = np.vstack([tab_as, np.full((1, HEADS), -1e30, np.float32)])
    tab_ad = np.vstack([tab_ad, np.zeros((1, HEADS), np.float32)])

    tab2_h = np.zeros((NROW, OUT), np.float32)
    tab2_as = np.zeros((NROW, 1), np.float32)
    tab2_ad = np.zeros((NROW, 1), np.float32)
    tab2_as[SENT] = -1e30
    W2e1f = hw["W2e1"].astype(np.float32)
    W2e2f = hw["W2e2"].astype(np.float32)
    b1f = hw["b1i"].astype(np.float32)[0]

    for c in range(NCORE):
        for t in range(T):
            d = int(d_t[t])
            rows = np.full((P, d), SENT, np.int64)
            for p in range(P):
                n = nid[c, t, p]
                if n != SENT:
                    k = int(deg[n])
                    rows[p, :k] = esrc[eptr[n]:eptr[n] + k]
            g_h = tab_h[rows]                       # [P,d,192] bf16-valued
            g_as = tab_as[rows]                     # [P,d,3] f32
            a_d = tab_ad[nid[c, t]]                 # [P,3]
            e = g_as + a_d[:, None, :]
            e = np.maximum(e, NEG_SLOPE * e)
            w = np.exp(e)                           # f32
            wb = _bf(w)
            s = w.sum(axis=1)                       # [P,3] f32 (reduce)
            prod = _bf(g_h * wb.repeat(HID, axis=2).reshape(P, d, HEADS, HID).transpose(0, 1, 3, 2).reshape(P, d, F1))
            # ^ interleaved: row pos i=(c*3+h) -> multiply by w[...,h]
            # tree-sum over d in bf16
            vals = [prod[:, j] for j in range(d)]
            while len(vals) > 1:
                nxt = []
                for j in range(0, len(vals) - 1, 2):
                    nxt.append(_bf(vals[j] + vals[j + 1]))
                if len(vals) % 2:
                    nxt.append(vals[-1])
                vals = nxt
            U = vals[0]                             # [P,192] bf16-valued
            r = _bf(1.0 / np.maximum(s, 1e-30))     # [P,3]
            h1 = _bf(U * np.repeat(r[:, None, :], HID, 1).reshape(P, F1))
            h1 = _bf(h1 + b1f)
            h1 = _bf(np.maximum(h1, 0) + _bf(np.exp(np.minimum(h1, 0))) - 1)
            # dense2
            out2 = _bf(h1) @ np.concatenate([W2e1f, W2e2f], 0)   # [P,66] f32 accum
            valid = nid[c, t] != SENT
            nn = nid[c, t][valid]
            tab2_h[nn] = _bf(out2[valid, :OUT])
            tab2_as[nn, 0] = out2[valid, OUT]
            tab2_ad[nn, 0] = out2[valid, OUT + 1]

    outp = np.zeros((N, OUT), np.float32)
    b2f = hw["b2b"][0]
    for c in range(NCORE):
        for t in range(T):
            d = int(d_t[t])
            rows = np.full((P, d), SENT, np.int64)
            for p in range(P):
                n = nid[c, t, p]
                if n != SENT:
                    k = int(deg[n])
                    rows[p, :k] = esrc[eptr[n]:eptr[n] + k]
            g_h = tab2_h[rows]                      # [P,d,64]
            g_as = tab2_as[rows]                    # [P,d,1]
            a_d = tab2_ad[nid[c, t]]                # [P,1]
            e = g_as + a_d[:, None, :]
            e = np.maximum(e, NEG_SLOPE * e)
            w = np.exp(e)
            wb = _bf(w)
            s = w.sum(axis=1)                       # [P,1]
            prod = _bf(g_h * wb)
            vals = [prod[:, j] for j in range(d)]
            while len(vals) > 1:
                nxt = []
                for j in range(0, len(vals) - 1, 2):
                    nxt.append(_bf(vals[j] + vals[j + 1]))
                if len(vals) % 2:
                    nxt.append(vals[-1])
                vals = nxt
            U2 = vals[0]
            r = 1.0 / np.maximum(s, 1e-30)
            o = U2 * r[:, None].reshape(P, 1) + b2f
            valid = nid[c, t] != SENT
            outp[nid[c, t][valid]] = o[valid]
    return outp


# ---------------------------------------------------------------- bass build
def _build_neff1(d_t, mul_gp_mod=0, chunk=CHUNK, pay_bufs=4):
    import concourse.bass as bass
    import concourse.bacc as bacc
    import concourse.mybir as mybir
    import concourse.tile as tile
    from concourse.masks import make_identity

    dt = mybir.dt
    nc = bacc.Bacc(num_swdge_queues=4)
    CE = int(8 * int(np.sum(d_t)))
    CN = NGRP * (G + 1) * 8
    xT = nc.dram_tensor("xT", [IN, NBLK * P], dt.bfloat16, kind="ExternalInput")
    W1e = nc.dram_tensor("W1e", [IN, 198], dt.bfloat16, kind="ExternalInput")
    W2e1 = nc.dram_tensor("W2e1", [128, 66], dt.bfloat16, kind="ExternalInput")
    W2e2 = nc.dram_tensor("W2e2", [64, 66], dt.bfloat16, kind="ExternalInput")
    b1i = nc.dram_tensor("b1i", [P, F1], dt.bfloat16, kind="ExternalInput")
    sent1 = nc.dram_tensor("sent1", [1, E1], dt.bfloat16, kind="ExternalInput")
    idxe = nc.dram_tensor("idxe", [P, CE], dt.int16, kind="ExternalInput")
    idxn = nc.dram_tensor("idxn", [P, CN], dt.int16, kind="ExternalInput")
    tab2loc = nc.dram_tensor("tab2loc", [T * P, 68], dt.bfloat16, kind="ExternalOutput")
    tab1 = nc.dram_tensor("tab1", [NROW, E1], dt.bfloat16)

    FQ = [0]

    def q():
        FQ[0] = (FQ[0] + 1) % 4
        return FQ[0]

    with tile.TileContext(nc) as tc:
        with tc.tile_pool(name="const", bufs=1) as cp, \
             tc.tile_pool(name="xp", bufs=3) as xp, \
             tc.tile_pool(name="rowp", bufs=3) as rowp, \
             tc.tile_pool(name="gp", bufs=pay_bufs) as gpool, \
             tc.tile_pool(name="np_", bufs=2) as npool, \
             tc.tile_pool(name="wp", bufs=3) as wpool, \
             tc.tile_pool(name="trp", bufs=2) as trp, \
             tc.tile_pool(name="hp", bufs=3) as hpool, \
             tc.tile_pool(name="psA", bufs=2, space="PSUM") as psA, \
             tc.tile_pool(name="psB", bufs=2, space="PSUM") as psB:

            w1_sb = cp.tile([IN, 198], dt.bfloat16)
            nc.sync.dma_start(out=w1_sb[:], in_=W1e[:, :])
            w2a_sb = cp.tile([128, 66], dt.bfloat16)
            nc.sync.dma_start(out=w2a_sb[:], in_=W2e1[:, :])
            w2b_sb = cp.tile([64, 66], dt.bfloat16)
            nc.sync.dma_start(out=w2b_sb[:], in_=W2e2[:, :])
            b1_sb = cp.tile([P, F1], dt.bfloat16)
            nc.sync.dma_start(out=b1_sb[:], in_=b1i[:, :])
            ide = cp.tile([P, P], dt.bfloat16)
            make_identity(nc, ide[:])
            ie_sb = cp.tile([P, CE], dt.int16)
            nc.sync.dma_start(out=ie_sb[:], in_=idxe[:, :])
            in_sb = cp.tile([P, CN], dt.int16)
            nc.sync.dma_start(out=in_sb[:], in_=idxn[:, :])
            sent_sb = cp.tile([1, E1], dt.bfloat16)
            nc.sync.dma_start(out=sent_sb[:], in_=sent1[:, :])
            nc.sync.dma_start(out=tab1[SENT:SENT + 1, :], in_=sent_sb[:])

            # ---------------- dense1: tab1 rows for all nodes
            XC = 4                        # x tiles per DMA chunk
            for ch in range((NBLK + XC - 1) // XC):
                ntile = min(XC, NBLK - ch * XC)
                xch = xp.tile([P, XC * P], dt.bfloat16, tag="x")
                nc.sync.dma_start(
                    out=xch[:, :ntile * P],
                    in_=xT[:, ch * XC * P: ch * XC * P + ntile * P])
                for k in range(ntile):
                    nt = ch * XC + k
                    pt = psA.tile([P, 198], dt.float32, tag="d1")
                    nc.tensor.matmul(pt[:], lhsT=xch[:, k * P:(k + 1) * P],
                                     rhs=w1_sb[:], start=True, stop=True)
                    rt = rowp.tile([P, E1], dt.bfloat16, tag="rt")
                    nc.scalar.activation(rt[:, :F1], pt[:, :F1],
                                         mybir.ActivationFunctionType.Copy)
                    nc.vector.tensor_copy(out=rt[:, F1:F1 + 12].bitcast(dt.float32),
                                          in_=pt[:, F1:198])
                    nrows = min(P, N - nt * P)
                    nc.sync.dma_start(out=tab1[nt * P: nt * P + nrows, :],
                                      in_=rt[:nrows, :])

            # ---------------- edge phase L1 + dense2
            tab_lo = tab1[BASE:, :]
            off_e = 0
            for g in range(NGRP):
                ndt = npool.tile([P, (G + 1) * E1], dt.bfloat16, tag="nd")
                nc.gpsimd.dma_gather(
                    out_ap=ndt[:].rearrange("p (j e) -> p j e", e=E1),
                    in_ap=tab_lo,
                    idxs_ap=in_sb[:, g * (G + 1) * 8:(g + 1) * (G + 1) * 8],
                    num_idxs=(G + 1) * P, num_idxs_reg=(G + 1) * P,
                    elem_size=E1, single_packet=False, queue_num=q())
                ndf = ndt[:].bitcast(dt.float32)       # [P, (G+1)*128]
                for j in range(G):
                    t = g * G + j
                    d = int(d_t[t])
                    pt = gpool.tile([P, d * E1], dt.bfloat16, tag="pay")
                    nch = (d + chunk - 1) // chunk
                    per = (d + nch - 1) // nch
                    s0 = 0
                    while s0 < d:
                        s1 = min(s0 + per, d)
                        nc.gpsimd.dma_gather(
                            out_ap=pt[:, s0 * E1:s1 * E1].rearrange("p (s e) -> p s e", e=E1),
                            in_ap=tab_lo,
                            idxs_ap=ie_sb[:, off_e + 8 * s0: off_e + 8 * s1],
                            num_idxs=(s1 - s0) * P, num_idxs_reg=(s1 - s0) * P,
                            elem_size=E1, single_packet=False, queue_num=q())
                        s0 = s1
                    off_e += 8 * d
                    ptf = pt[:].bitcast(dt.float32)    # [P, d*128] f32 view
                    a_s = ptf.rearrange("p (s e) -> p s e", e=128)[:, :, 96:99]
                    a_d = ndf.rearrange("p (j e) -> p j e", e=128)[:, j, 99:102]

                    et = wpool.tile([P, d * 3], dt.float32, tag="e")
                    nc.vector.tensor_tensor(
                        out=et[:].rearrange("p (s h) -> p s h", h=3),
                        in0=a_s, in1=a_d.unsqueeze(1).to_broadcast([P, d, 3]),
                        op=mybir.AluOpType.add)
                    et2 = wpool.tile([P, d * 3], dt.float32, tag="e2")
                    nc.vector.scalar_tensor_tensor(
                        out=et2[:], in0=et[:], scalar=NEG_SLOPE, in1=et[:],
                        op0=mybir.AluOpType.mult, op1=mybir.AluOpType.max)
                    wf = wpool.tile([P, d * 3], dt.float32, tag="wf")
                    nc.scalar.activation(wf[:], et2[:], mybir.ActivationFunctionType.Exp)
                    wb = wpool.tile([P, d * 3], dt.bfloat16, tag="wb")
                    nc.vector.tensor_copy(out=wb[:], in_=wf[:])
                    st = wpool.tile([P, 3], dt.float32, tag="s")
                    nc.vector.tensor_reduce(
                        out=st[:],
                        in_=wf[:].rearrange("p (s h) -> p h s", h=3),
                        axis=mybir.AxisListType.X, op=mybir.AluOpType.add)
                    # prod = h * w, in place in pt (all operands innermost step-1)
                    h_view = pt[:].rearrange("p (s e) -> p s e", e=E1)[:, :, :F1] \
                                  .rearrange("p s (c h) -> p s c h", h=3)
                    w_view = wb[:].rearrange("p (s h) -> p s h", h=3) \
                                  .unsqueeze(2).to_broadcast([P, d, HID, 3])
                    eng = nc.gpsimd if (mul_gp_mod and t % mul_gp_mod == mul_gp_mod - 1) else nc.vector
                    eng.tensor_tensor(out=h_view, in0=h_view, in1=w_view,
                                      op=mybir.AluOpType.mult)
                    # pairwise tree over slots; level 1 reads pt (stride E1)
                    cur, n = pt, d
                    stride = E1
                    lvl = 0
                    while n > 1:
                        half, odd = n // 2, n % 2
                        dst_t = trp.tile([P, (half + odd) * F1], dt.bfloat16,
                                         tag=f"tr{1 + (lvl % 2)}")
                        slots = cur[:, :n * stride].rearrange(
                            "p (s e) -> p s e", e=stride)[:, :, :F1]
                        nc.vector.tensor_tensor(
                            out=dst_t[:].rearrange("p (s f) -> p s f", f=F1)[:, :half],
                            in0=slots[:, 0:2 * half:2],
                            in1=slots[:, 1:2 * half:2],
                            op=mybir.AluOpType.add)
                        if odd:
                            nc.vector.tensor_copy(
                                out=dst_t[:, half * F1:(half + 1) * F1],
                                in_=cur[:, (n - 1) * stride:(n - 1) * stride + F1])
                        cur, n = dst_t, half + odd
                        stride = F1
                        lvl += 1
                    # normalize + bias + ELU (bf16)
                    sc = wpool.tile([P, 3], dt.float32, tag="sc")
                    nc.vector.tensor_scalar_max(out=sc[:], in0=st[:], scalar1=1e-30)
                    rc = wpool.tile([P, 3], dt.float32, tag="rc")
                    nc.vector.reciprocal(out=rc[:], in_=sc[:])
                    rcb = wpool.tile([P, 3], dt.bfloat16, tag="rcb")
                    nc.vector.tensor_copy(out=rcb[:], in_=rc[:])
                    h1 = hpool.tile([P, F1], dt.bfloat16, tag="h1")
                    nc.vector.tensor_tensor(
                        out=h1[:].rearrange("p (c h) -> p c h", h=3),
                        in0=cur[:, :F1].rearrange("p (c h) -> p c h", h=3),
                        in1=rcb[:].unsqueeze(1).to_broadcast([P, HID, 3]),
                        op=mybir.AluOpType.mult)
                    nc.vector.tensor_add(out=h1[:], in0=h1[:], in1=b1_sb[:])
                    tmin = hpool.tile([P, F1], dt.bfloat16, tag="tmin")
                    nc.vector.tensor_scalar_min(out=tmin[:], in0=h1[:], scalar1=0.0)
                    texp = hpool.tile([P, F1], dt.bfloat16, tag="texp")
                    nc.scalar.activation(texp[:], tmin[:], mybir.ActivationFunctionType.Exp)
                    h1e = hpool.tile([P, F1], dt.bfloat16, tag="h1e")
                    nc.vector.scalar_tensor_tensor(
                        out=h1e[:], in0=h1[:], scalar=0.0, in1=texp[:],
                        op0=mybir.AluOpType.max, op1=mybir.AluOpType.add)
                    nc.vector.tensor_scalar_add(out=h1e[:], in0=h1e[:], scalar1=-1.0)
                    # transpose 192 = 128 + 64
                    tp1 = psB.tile([P, P], dt.bfloat16, tag="tp1")
                    nc.tensor.transpose(tp1[:], h1e[:, :P], ide[:])
                    tp2 = psB.tile([64, P], dt.bfloat16, tag="tp2")
                    nc.tensor.transpose(tp2[:], h1e[:, P:F1], ide[:])
                    hT1 = hpool.tile([P, P], dt.bfloat16, tag="hT1")
                    nc.vector.tensor_copy(out=hT1[:], in_=tp1[:])
                    hT2 = hpool.tile([64, P], dt.bfloat16, tag="hT2")
                    nc.vector.tensor_copy(out=hT2[:], in_=tp2[:])
                    o2 = psA.tile([P, 66], dt.float32, tag="o2")
                    nc.tensor.matmul(o2[:], lhsT=hT1[:], rhs=w2a_sb[:], start=True, stop=False)
                    nc.tensor.matmul(o2[:], lhsT=hT2[:], rhs=w2b_sb[:], start=False, stop=True)
                    r2 = rowp.tile([P, 68], dt.bfloat16, tag="r2")
                    nc.scalar.activation(r2[:, :OUT], o2[:, :OUT],
                                         mybir.ActivationFunctionType.Copy)
                    nc.vector.tensor_copy(out=r2[:, OUT:OUT + 4].bitcast(dt.float32),
                                          in_=o2[:, OUT:OUT + 2])
                    nc.sync.dma_start(out=tab2loc[t * P:(t + 1) * P, :], in_=r2[:])
    nc.compile()
    return nc


def _build_neff2(d_t, mul_gp_mod=0, chunk=CHUNK, pay_bufs=6):
    import concourse.bass as bass
    import concourse.bacc as bacc
    import concourse.mybir as mybir
    import concourse.tile as tile

    dt = mybir.dt
    nc = bacc.Bacc(num_swdge_queues=4)
    CE = int(8 * int(np.sum(d_t)))
    CN = NGRP * (G + 1) * 8
    tab2 = nc.dram_tensor("tab2", [NROW, E2], dt.bfloat16, kind="ExternalInput")
    idxe = nc.dram_tensor("idxe", [P, CE], dt.int16, kind="ExternalInput")
    idxn = nc.dram_tensor("idxn", [P, CN], dt.int16, kind="ExternalInput")
    b2b = nc.dram_tensor("b2b", [P, OUT], dt.float32, kind="ExternalInput")
    out2 = nc.dram_tensor("out2", [T * P, OUT], dt.float32, kind="ExternalOutput")

    FQ = [0]

    def q():
        FQ[0] = (FQ[0] + 1) % 4
        return FQ[0]

    with tile.TileContext(nc) as tc:
        with tc.tile_pool(name="const", bufs=1) as cp, \
             tc.tile_pool(name="gp", bufs=pay_bufs) as gpool, \
             tc.tile_pool(name="np_", bufs=2) as npool, \
             tc.tile_pool(name="wp", bufs=3) as wpool, \
             tc.tile_pool(name="trp", bufs=2) as trp, \
             tc.tile_pool(name="op", bufs=3) as opool:
            ie_sb = cp.tile([P, CE], dt.int16)
            nc.sync.dma_start(out=ie_sb[:], in_=idxe[:, :])
            in_sb = cp.tile([P, CN], dt.int16)
            nc.sync.dma_start(out=in_sb[:], in_=idxn[:, :])
            b2_sb = cp.tile([P, OUT], dt.float32)
            nc.sync.dma_start(out=b2_sb[:], in_=b2b[:, :])

            tab_lo = tab2[BASE:, :]
            off_e = 0
            for g in range(NGRP):
                ndt = npool.tile([P, (G + 1) * E2], dt.bfloat16, tag="nd")
                nc.gpsimd.dma_gather(
                    out_ap=ndt[:].rearrange("p (j e) -> p j e", e=E2),
                    in_ap=tab_lo,
                    idxs_ap=in_sb[:, g * (G + 1) * 8:(g + 1) * (G + 1) * 8],
                    num_idxs=(G + 1) * P, num_idxs_reg=(G + 1) * P,
                    elem_size=E2, single_packet=False, queue_num=q())
                ndf = ndt[:].bitcast(dt.float32)     # [P, (G+1)*64]
                for j in range(G):
                    t = g * G + j
                    d = int(d_t[t])
                    pt = gpool.tile([P, d * E2], dt.bfloat16, tag="pay")
                    nch = (d + chunk - 1) // chunk
                    per = (d + nch - 1) // nch
                    s0 = 0
                    while s0 < d:
                        s1 = min(s0 + per, d)
                        nc.gpsimd.dma_gather(
                            out_ap=pt[:, s0 * E2:s1 * E2].rearrange("p (s e) -> p s e", e=E2),
                            in_ap=tab_lo,
                            idxs_ap=ie_sb[:, off_e + 8 * s0: off_e + 8 * s1],
                            num_idxs=(s1 - s0) * P, num_idxs_reg=(s1 - s0) * P,
                            elem_size=E2, single_packet=False, queue_num=q())
                        s0 = s1
                    off_e += 8 * d
                    ptf = pt[:].bitcast(dt.float32)   # [P, d*64]
                    a_s = ptf.rearrange("p (s e) -> p s e", e=64)[:, :, 32:33]
                    a_d = ndf.rearrange("p (j e) -> p j e", e=64)[:, j, 33:34]
                    et = wpool.tile([P, d], dt.float32, tag="e")
                    nc.vector.tensor_tensor(
                        out=et[:].unsqueeze(2),
                        in0=a_s, in1=a_d.unsqueeze(1).to_broadcast([P, d, 1]),
                        op=mybir.AluOpType.add)
                    et2 = wpool.tile([P, d], dt.float32, tag="e2")
                    nc.vector.scalar_tensor_tensor(
                        out=et2[:], in0=et[:], scalar=NEG_SLOPE, in1=et[:],
                        op0=mybir.AluOpType.mult, op1=mybir.AluOpType.max)
                    wf = wpool.tile([P, d], dt.float32, tag="wf")
                    nc.scalar.activation(wf[:], et2[:], mybir.ActivationFunctionType.Exp)
                    wb = wpool.tile([P, d], dt.bfloat16, tag="wb")
                    nc.vector.tensor_copy(out=wb[:], in_=wf[:])
                    st = wpool.tile([P, 1], dt.float32, tag="s")
                    nc.vector.tensor_reduce(out=st[:], in_=wf[:],
                                            axis=mybir.AxisListType.X,
                                            op=mybir.AluOpType.add)
                    h_view = pt[:].rearrange("p (s e) -> p s e", e=E2)[:, :, :OUT]
                    w_view = wb[:].unsqueeze(2).to_broadcast([P, d, OUT])
                    eng = nc.gpsimd if (mul_gp_mod and t % mul_gp_mod == mul_gp_mod - 1) else nc.vector
                    eng.tensor_tensor(out=h_view, in0=h_view, in1=w_view,
                                      op=mybir.AluOpType.mult)
                    cur, n = pt, d
                    stride = E2
                    lvl = 0
                    while n > 1:
                        half, odd = n // 2, n % 2
                        dst_t = trp.tile([P, (half + odd) * OUT], dt.bfloat16,
                                         tag=f"tr{1 + (lvl % 2)}")
                        nc.vector.tensor_tensor(
                            out=dst_t[:].rearrange("p (s f) -> p s f", f=OUT)[:, :half],
                            in0=cur[:].rearrange("p (s e) -> p s e", e=stride)[:, 0:2 * half:2, :OUT],
                            in1=cur[:].rearrange("p (s e) -> p s e", e=stride)[:, 1:2 * half:2, :OUT],
                            op=mybir.AluOpType.add)
                        if odd:
                            nc.vector.tensor_copy(
                                out=dst_t[:, half * OUT:(half + 1) * OUT],
                                in_=cur[:, (n - 1) * stride:(n - 1) * stride + OUT])
                        cur, n = dst_t, half + odd
                        stride = OUT
                        lvl += 1
                    sc = wpool.tile([P, 1], dt.float32, tag="sc")
                    nc.vector.tensor_scalar_max(out=sc[:], in0=st[:], scalar1=1e-30)
                    rc = wpool.tile([P, 1], dt.float32, tag="rc")
                    nc.vector.reciprocal(out=rc[:], in_=sc[:])
                    ot = opool.tile([P, OUT], dt.float32, tag="ot")
                    nc.vector.tensor_scalar(
                        out=ot[:], in0=cur[:, :OUT], scalar1=rc[:, :1], scalar2=None,
                        op0=mybir.AluOpType.mult)
                    nc.vector.tensor_add(out=ot[:], in0=ot[:], in1=b2_sb[:])
                    nc.sync.dma_start(out=out2[t * P:(t + 1) * P, :], in_=ot[:])
    nc.compile()
    return nc


# ---------------------------------------------------------------- kernel
def kernel(x, edge_index, W1, att_src1, att_dst1, b1, W2, att_src2, att_dst2, b2,
           _emulate=False, _timing=None):
    x = np.asarray(x, np.float32)
    edge_index = np.asarray(edge_index)
    W1 = np.asarray(W1, np.float32)
    att_src1 = np.asarray(att_src1, np.float32)
    att_dst1 = np.asarray(att_dst1, np.float32)
    b1 = np.asarray(b1, np.float32)
    W2 = np.asarray(W2, np.float32)
    att_src2 = np.asarray(att_src2, np.float32)
    att_dst2 = np.asarray(att_dst2, np.float32)
    b2 = np.asarray(b2, np.float32)

    if _emulate:
        return emulate(x, edge_index, W1, att_src1, att_dst1, b1,
                       W2, att_src2, att_dst2, b2)

    from concourse.bass_utils import run_bass_kernel_spmd
    import time as _time

    def _run(nc, maps, trace):
        try:
            return run_bass_kernel_spmd(nc, maps, core_ids=list(range(NCORE)), trace=trace)
        except Exception:
            _time.sleep(25)
            return run_bass_kernel_spmd(nc, maps, core_ids=list(range(NCORE)), trace=False)

    pre = preprocess(edge_index)
    hw = host_weights(x, W1, att_src1, att_dst1, b1, W2, att_src2, att_dst2, b2)
    d_t, nid = pre["d_t"], pre["nid"]

    trace = _timing is not None

    # ---- NEFF1
    nc1 = _build_neff1(d_t)
    maps1 = [dict(xT=hw["xT"], W1e=hw["W1e"], W2e1=hw["W2e1"], W2e2=hw["W2e2"],
                  b1i=hw["b1i"], sent1=hw["sent1"],
                  idxe=pre["idxe"][c], idxn=pre["idxn"][c]) for c in range(NCORE)]
    res1 = _run(nc1, maps1, trace)

    # host: assemble global table2
    tab2 = np.zeros((NROW, E2), bf16)
    tab2[SENT].view(np.float32)[32] = -1e30
    for c in range(NCORE):
        loc = res1.results[c]["tab2loc"]           # [T*P, 68] bf16
        nn = nid[c].reshape(-1)                    # [T*P]
        valid = nn != SENT
        tab2[nn[valid], :68] = loc[valid]

    # ---- NEFF2
    nc2 = _build_neff2(d_t)
    maps2 = [dict(tab2=tab2, idxe=pre["idxe"][c], idxn=pre["idxn"][c],
                  b2b=hw["b2b"]) for c in range(NCORE)]
    res2 = _run(nc2, maps2, trace)

    out = np.zeros((N, OUT), np.float32)
    for c in range(NCORE):
        o = res2.results[c]["out2"]
        nn = nid[c].reshape(-1)
        valid = nn != SENT
        out[nn[valid]] = o[valid]

    if _timing is not None:
        _timing["neff1_ns"] = res1.exec_time_ns
        _timing["neff2_ns"] = res2.exec_time_ns
    return out


# revision 8
# speedup vs baseline: 1.9724x; 1.2787x over previous
"""2-layer GAT (PyG-style) on TRN2, 8 NeuronCores.

Strategy: degree-bucketed node layout. Nodes sorted by in-degree (desc),
grouped into 128-node blocks; block b -> core b%8, tile t=b//8 (49 tiles/core).
Each tile is [128 node-partitions x d_t edge slots] with per-tile uniform
slot count (padding to sentinel). Per-edge source features are fetched with
dma_gather (custom GPSIMD bulk gather; signed int16 indices against a
mid-table base row 32768). The segment softmax + weighted aggregation are
pure row-wise DVE ops (multiply + pairwise-tree sum along the free dim).

Layer-1 table rows (bf16, 512B): [h (192, (c,h)-interleaved) | a_s 3xf32 |
a_d 3xf32 | pad]. Interleaving features as (c,h) keeps every operand of the
big per-edge multiply at innermost stride 1 => DVE 2x mode.
Layer-2 table rows (bf16, 256B): [h2 (64) | a_s2 f32 | a_d2 f32 | pad].

Softmax uses exp(leaky_relu(a_s+a_d)) without max-subtraction (logits are
O(10), no overflow in f32), normalized once per node: out = U / max(s,tiny).

Two NEFF launches: NEFF1 = dense1 (full, redundant per core) + L1 edge
phase + normalize/ELU + dense2 -> per-core table2 shard. Host assembles the
global table2. NEFF2 = L2 edge phase -> output shard. Host unpermutes.
"""
import numpy as np
import ml_dtypes

N = 50000
E = 800000
IN = 128
HID = 64
HEADS = 3
OUT = 64
NCORE = 8
P = 128
NBLK = (N + P - 1) // P            # 391
NBLKP = ((NBLK + NCORE - 1) // NCORE) * NCORE  # 392
T = NBLKP // NCORE                 # 49 tiles per core
G = 7                              # tiles per nid-gather group
NGRP = T // G                      # 7
SENT = N                           # sentinel table row
NROW = N + 1
BASE = 32768                       # gather base row (signed int16 trick)
E1 = 256                           # L1 row elems (bf16)
E2 = 128                           # L2 row elems (bf16)
NEG_SLOPE = 0.2
F1 = HEADS * HID                   # 192
CHUNK = 8                          # max slots per dma_gather call

bf16 = ml_dtypes.bfloat16

# interleaved feature order: row position i=(c*3+h) holds feature f=h*64+c
_POS = np.arange(F1)
PERM = (_POS % HEADS) * HID + (_POS // HEADS)   # row pos i -> original feature


def _pack_idx(rows_flat):
    """rows_flat int64[nidx] (table rows, nidx%128==0) -> int16 [128, nidx//16]
    wrap-16 layout replicated across the 8 groups of 16 partitions."""
    v = (rows_flat - BASE).astype(np.int16)
    cid = len(v) // 16
    block = v.reshape(cid, 16).T
    return np.tile(block, (8, 1))


def preprocess(edge_index):
    src = np.concatenate([edge_index[0].astype(np.int64), np.arange(N, dtype=np.int64)])
    dst = np.concatenate([edge_index[1].astype(np.int64), np.arange(N, dtype=np.int64)])
    deg = np.bincount(dst, minlength=N)
    order = np.argsort(-deg, kind="stable")          # node processing order
    eorder = np.argsort(dst, kind="stable")
    esrc = src[eorder]                               # edge srcs grouped by dst
    eptr = np.zeros(N + 1, np.int64)
    eptr[1:] = np.cumsum(deg)

    # per-tile slot counts (shared across cores - SPMD).
    # Partition 127 needs its chunk-boundary-last slots to be sentinel pads
    # (the gather drops trailing-negative indices per call), so ensure
    # deg(p127) <= d - n_chunks.
    d_t = []
    for t in range(T):
        g0 = (t * NCORE) * P
        dmax = int(deg[order[g0]]) if g0 < N else 2
        d127 = 0
        for c in range(NCORE):
            gi = (t * NCORE + c) * P + (P - 1)
            if gi < N:
                d127 = max(d127, int(deg[order[gi]]))
        d = max(dmax, 2)
        while True:
            nch = (d + CHUNK - 1) // CHUNK
            need = max(dmax, d127 + nch, 2)
            if need <= d:
                break
            d = need
        d_t.append(d)
    d_t = np.array(d_t, np.int64)

    # node grid per core: nid[c][t, p] = node id or SENT
    gidx = (np.arange(T)[:, None] * NCORE)[None] * P  # placeholder
    nid = np.full((NCORE, T, P), SENT, np.int64)
    for c in range(NCORE):
        gi = ((np.arange(T) * NCORE + c)[:, None]) * P + np.arange(P)[None, :]
        valid = gi < N
        nid[c][valid] = order[gi[valid]]

    # edge slot grids + packed indices
    idxe = np.zeros((NCORE, P, int(8 * d_t.sum())), np.int16)
    idxn = np.zeros((NCORE, P, NGRP * (G + 1) * 8), np.int16)
    for c in range(NCORE):
        eblocks = []
        for t in range(T):
            d = int(d_t[t])
            rows = np.full((P, d), SENT, np.int64)
            nch = (d + CHUNK - 1) // CHUNK
            per = (d + nch - 1) // nch
            bnd = set()
            s0 = 0
            while s0 < d:
                s1 = min(s0 + per, d)
                bnd.add(s1 - 1)
                s0 = s1
            p127_slots = [s for s in range(d) if s not in bnd]
            for p in range(P):
                n = nid[c, t, p]
                if n != SENT:
                    k = int(deg[n])
                    if p == P - 1:
                        assert k <= len(p127_slots)
                        rows[p, p127_slots[:k]] = esrc[eptr[n]:eptr[n] + k]
                    else:
                        rows[p, :k] = esrc[eptr[n]:eptr[n] + k]
            eblocks.append(_pack_idx(rows.T.reshape(-1)))
        idxe[c] = np.concatenate(eblocks, axis=1)
        nblocks = []
        for g in range(NGRP):
            rows = np.full((P, G + 1), SENT, np.int64)
            for j in range(G):
                rows[:, j] = nid[c, g * G + j]
            nblocks.append(_pack_idx(rows.T.reshape(-1)))
        idxn[c] = np.concatenate(nblocks, axis=1)

    return dict(order=order, nid=nid, d_t=d_t, idxe=idxe, idxn=idxn,
                deg=deg, esrc=esrc, eptr=eptr)


def host_weights(x, W1, att_src1, att_dst1, b1, W2, att_src2, att_dst2, b2):
    W1s = np.stack([W1[:, h * HID:(h + 1) * HID] @ att_src1[h] for h in range(HEADS)], 1)  # [128,3]
    W1d = np.stack([W1[:, h * HID:(h + 1) * HID] @ att_dst1[h] for h in range(HEADS)], 1)
    W1e = np.concatenate([W1[:, PERM], W1s, W1d], axis=1)           # [128,198]
    W2e = np.concatenate([W2, (W2 @ att_src2[0])[:, None], (W2 @ att_dst2[0])[:, None]], 1)  # [192,66]
    W2e = W2e[PERM, :]                                               # rows in interleaved order
    xT = np.zeros((IN, NBLK * P), np.float32)
    xT[:, :N] = x.T
    sent1 = np.zeros(E1, bf16)
    sent1.view(np.float32)[96:99] = -1e30   # a_s = -inf-ish
    b1i = b1[PERM].astype(bf16)
    return dict(xT=xT.astype(bf16), W1e=W1e.astype(bf16),
                W2e1=W2e[:128].astype(bf16), W2e2=W2e[128:].astype(bf16),
                sent1=sent1.reshape(1, E1), b1i=np.tile(b1i, (P, 1)),
                b2b=np.tile(b2.astype(np.float32), (P, 1)))


# ---------------------------------------------------------------- emulation
def _bf(a):
    return a.astype(bf16).astype(np.float32)


def emulate(x, edge_index, W1, att_src1, att_dst1, b1, W2, att_src2, att_dst2, b2):
    """numpy emulation of the device pipeline (bf16 rounding where the HW has it)."""
    pre = preprocess(edge_index)
    hw = host_weights(x, W1, att_src1, att_dst1, b1, W2, att_src2, att_dst2, b2)
    d_t, nid = pre["d_t"], pre["nid"]
    deg, esrc, eptr = pre["deg"], pre["esrc"], pre["eptr"]

    # dense1 (as the device does: bf16 x, bf16 W1e, f32 accum)
    xTf = hw["xT"].astype(np.float32)[:, :N]
    W1ef = hw["W1e"].astype(np.float32)
    H = xTf.T @ W1ef                      # [N, 198] f32
    tab_h = _bf(H[:, :F1])                # bf16 h (interleaved)
    tab_as = H[:, 192:195]
    tab_ad = H[:, 195:198]
    # sentinel
    tab_h = np.vstack([tab_h, np.zeros((1, F1), np.float32)])
    tab_as = np.vstack([tab_as, np.full((1, HEADS), -1e30, np.float32)])
    tab_ad = np.vstack([tab_ad, np.zeros((1, HEADS), np.float32)])

    tab2_h = np.zeros((NROW, OUT), np.float32)
    tab2_as = np.zeros((NROW, 1), np.float32)
    tab2_ad = np.zeros((NROW, 1), np.float32)
    tab2_as[SENT] = -1e30
    W2e1f = hw["W2e1"].astype(np.float32)
    W2e2f = hw["W2e2"].astype(np.float32)
    b1f = hw["b1i"].astype(np.float32)[0]

    for c in range(NCORE):
        for t in range(T):
            d = int(d_t[t])
            rows = np.full((P, d), SENT, np.int64)
            for p in range(P):
                n = nid[c, t, p]
                if n != SENT:
                    k = int(deg[n])
                    rows[p, :k] = esrc[eptr[n]:eptr[n] + k]
            g_h = tab_h[rows]                       # [P,d,192] bf16-valued
            g_as = tab_as[rows]                     # [P,d,3] f32
            a_d = tab_ad[nid[c, t]]                 # [P,3]
            e = g_as + a_d[:, None, :]
            e = np.maximum(e, NEG_SLOPE * e)
            w = np.exp(e)                           # f32
            wb = _bf(w)
            s = w.sum(axis=1)                       # [P,3] f32 (reduce)
            prod = _bf(g_h * wb.repeat(HID, axis=2).reshape(P, d, HEADS, HID).transpose(0, 1, 3, 2).reshape(P, d, F1))
            # ^ interleaved: row pos i=(c*3+h) -> multiply by w[...,h]
            # tree-sum over d in bf16
            vals = [prod[:, j] for j in range(d)]
            while len(vals) > 1:
                nxt = []
                for j in range(0, len(vals) - 1, 2):
                    nxt.append(_bf(vals[j] + vals[j + 1]))
                if len(vals) % 2:
                    nxt.append(vals[-1])
                vals = nxt
            U = vals[0]                             # [P,192] bf16-valued
            r = _bf(1.0 / np.maximum(s, 1e-30))     # [P,3]
            h1 = _bf(U * np.repeat(r[:, None, :], HID, 1).reshape(P, F1))
            h1 = _bf(h1 + b1f)
            h1 = _bf(np.maximum(h1, 0) + _bf(np.exp(np.minimum(h1, 0))) - 1)
            # dense2
            out2 = _bf(h1) @ np.concatenate([W2e1f, W2e2f], 0)   # [P,66] f32 accum
            valid = nid[c, t] != SENT
            nn = nid[c, t][valid]
            tab2_h[nn] = _bf(out2[valid, :OUT])
            tab2_as[nn, 0] = out2[valid, OUT]
            tab2_ad[nn, 0] = out2[valid, OUT + 1]

    outp = np.zeros((N, OUT), np.float32)
    b2f = hw["b2b"][0]
    for c in range(NCORE):
        for t in range(T):
            d = int(d_t[t])
            rows = np.full((P, d), SENT, np.int64)
            for p in range(P):
                n = nid[c, t, p]
                if n != SENT:
                    k = int(deg[n])
                    rows[p, :k] = esrc[eptr[n]:eptr[n] + k]
            g_h = tab2_h[rows]                      # [P,d,64]
            g_as = tab2_as[rows]                    # [P,d,1]
            a_d = tab2_ad[nid[c, t]]                # [P,1]
            e = g_as + a_d[:, None, :]
            e = np.maximum(e, NEG_SLOPE * e)
            w = np.exp(e)
            wb = _bf(w)
            s = w.sum(axis=1)                       # [P,1]
            prod = _bf(g_h * wb)
            vals = [prod[:, j] for j in range(d)]
            while len(vals) > 1:
                nxt = []
                for j in range(0, len(vals) - 1, 2):
                    nxt.append(_bf(vals[j] + vals[j + 1]))
                if len(vals) % 2:
                    nxt.append(vals[-1])
                vals = nxt
            U2 = vals[0]
            r = 1.0 / np.maximum(s, 1e-30)
            o = U2 * r[:, None].reshape(P, 1) + b2f
            valid = nid[c, t] != SENT
            outp[nid[c, t][valid]] = o[valid]
    return outp


# ---------------------------------------------------------------- bass build
def _build_neff1(d_t, mul_gp_mod=0, chunk=CHUNK, pay_bufs=4):
    import concourse.bass as bass
    import concourse.bacc as bacc
    import concourse.mybir as mybir
    import concourse.tile as tile
    from concourse.masks import make_identity

    dt = mybir.dt
    nc = bacc.Bacc(num_swdge_queues=4)
    CE = int(8 * int(np.sum(d_t)))
    CN = NGRP * (G + 1) * 8
    xT = nc.dram_tensor("xT", [IN, NBLK * P], dt.bfloat16, kind="ExternalInput")
    W1e = nc.dram_tensor("W1e", [IN, 198], dt.bfloat16, kind="ExternalInput")
    W2e1 = nc.dram_tensor("W2e1", [128, 66], dt.bfloat16, kind="ExternalInput")
    W2e2 = nc.dram_tensor("W2e2", [64, 66], dt.bfloat16, kind="ExternalInput")
    b1i = nc.dram_tensor("b1i", [P, F1], dt.bfloat16, kind="ExternalInput")
    sent1 = nc.dram_tensor("sent1", [1, E1], dt.bfloat16, kind="ExternalInput")
    idxe = nc.dram_tensor("idxe", [P, CE], dt.int16, kind="ExternalInput")
    idxn = nc.dram_tensor("idxn", [P, CN], dt.int16, kind="ExternalInput")
    tab2loc = nc.dram_tensor("tab2loc", [T * P, 68], dt.bfloat16, kind="ExternalOutput")
    tab1 = nc.dram_tensor("tab1", [NROW, E1], dt.bfloat16)

    FQ = [0]

    def q():
        FQ[0] = (FQ[0] + 1) % 4
        return FQ[0]

    with tile.TileContext(nc) as tc:
        with tc.tile_pool(name="const", bufs=1) as cp, \
             tc.tile_pool(name="xp", bufs=3) as xp, \
             tc.tile_pool(name="rowp", bufs=3) as rowp, \
             tc.tile_pool(name="gp", bufs=pay_bufs) as gpool, \
             tc.tile_pool(name="np_", bufs=2) as npool, \
             tc.tile_pool(name="wp", bufs=3) as wpool, \
             tc.tile_pool(name="trp", bufs=2) as trp, \
             tc.tile_pool(name="hp", bufs=3) as hpool, \
             tc.tile_pool(name="psA", bufs=2, space="PSUM") as psA, \
             tc.tile_pool(name="psB", bufs=2, space="PSUM") as psB:

            w1_sb = cp.tile([IN, 198], dt.bfloat16)
            nc.sync.dma_start(out=w1_sb[:], in_=W1e[:, :])
            w2a_sb = cp.tile([128, 66], dt.bfloat16)
            nc.sync.dma_start(out=w2a_sb[:], in_=W2e1[:, :])
            w2b_sb = cp.tile([64, 66], dt.bfloat16)
            nc.sync.dma_start(out=w2b_sb[:], in_=W2e2[:, :])
            b1_sb = cp.tile([P, F1], dt.bfloat16)
            nc.sync.dma_start(out=b1_sb[:], in_=b1i[:, :])
            ide = cp.tile([P, P], dt.bfloat16)
            make_identity(nc, ide[:])
            ie_sb = cp.tile([P, CE], dt.int16)
            nc.sync.dma_start(out=ie_sb[:], in_=idxe[:, :])
            in_sb = cp.tile([P, CN], dt.int16)
            nc.sync.dma_start(out=in_sb[:], in_=idxn[:, :])
            sent_sb = cp.tile([1, E1], dt.bfloat16)
            nc.sync.dma_start(out=sent_sb[:], in_=sent1[:, :])
            nc.sync.dma_start(out=tab1[SENT:SENT + 1, :], in_=sent_sb[:])

            # ---------------- dense1: tab1 rows for all nodes
            XC = 8                        # x tiles per DMA chunk / table tiles per write
            for ch in range((NBLK + XC - 1) // XC):
                ntile = min(XC, NBLK - ch * XC)
                xch = xp.tile([P, XC * P], dt.bfloat16, tag="x")
                nc.sync.dma_start(
                    out=xch[:, :ntile * P],
                    in_=xT[:, ch * XC * P: ch * XC * P + ntile * P])
                rt = rowp.tile([P, XC * E1], dt.bfloat16, tag="rt")
                for k in range(ntile):
                    nt = ch * XC + k
                    pt = psA.tile([P, 198], dt.float32, tag="d1")
                    nc.tensor.matmul(pt[:], lhsT=xch[:, k * P:(k + 1) * P],
                                     rhs=w1_sb[:], start=True, stop=True)
                    nc.scalar.activation(rt[:, k * E1:k * E1 + F1], pt[:, :F1],
                                         mybir.ActivationFunctionType.Copy)
                    nc.vector.tensor_copy(
                        out=rt[:, k * E1 + F1:k * E1 + F1 + 12].bitcast(dt.float32),
                        in_=pt[:, F1:198])
                # one DMA for up to XC tiles: row (nt, p) -> tab1[nt*P + p]
                full = min(ntile * P, N - ch * XC * P)
                dst = tab1[ch * XC * P: ch * XC * P + full, :]
                if full == ntile * P:
                    dst_v = dst.rearrange("(k p) e -> p k e", p=P)
                    src_v = rt[:, :ntile * E1].rearrange("p (k e) -> p k e", e=E1)
                    nc.sync.dma_start(out=dst_v, in_=src_v)
                else:
                    # last chunk: partial rows; fall back to per-tile writes
                    for k in range(ntile):
                        nt = ch * XC + k
                        nrows = min(P, N - nt * P)
                        if nrows > 0:
                            nc.sync.dma_start(
                                out=tab1[nt * P: nt * P + nrows, :],
                                in_=rt[:nrows, k * E1:(k + 1) * E1])

            # ---------------- edge phase L1 + dense2
            tab_lo = tab1[BASE:, :]
            off_e = 0
            for g in range(NGRP):
                ndt = npool.tile([P, (G + 1) * E1], dt.bfloat16, tag="nd")
                nc.gpsimd.dma_gather(
                    out_ap=ndt[:].rearrange("p (j e) -> p j e", e=E1),
                    in_ap=tab_lo,
                    idxs_ap=in_sb[:, g * (G + 1) * 8:(g + 1) * (G + 1) * 8],
                    num_idxs=(G + 1) * P, num_idxs_reg=(G + 1) * P,
                    elem_size=E1, single_packet=False, queue_num=q())
                ndf = ndt[:].bitcast(dt.float32)       # [P, (G+1)*128]
                for j in range(G):
                    t = g * G + j
                    d = int(d_t[t])
                    pt = gpool.tile([P, d * E1], dt.bfloat16, tag="pay")
                    nch = (d + chunk - 1) // chunk
                    per = (d + nch - 1) // nch
                    s0 = 0
                    while s0 < d:
                        s1 = min(s0 + per, d)
                        nc.gpsimd.dma_gather(
                            out_ap=pt[:, s0 * E1:s1 * E1].rearrange("p (s e) -> p s e", e=E1),
                            in_ap=tab_lo,
                            idxs_ap=ie_sb[:, off_e + 8 * s0: off_e + 8 * s1],
                            num_idxs=(s1 - s0) * P, num_idxs_reg=(s1 - s0) * P,
                            elem_size=E1, single_packet=False, queue_num=q())
                        s0 = s1
                    off_e += 8 * d
                    ptf = pt[:].bitcast(dt.float32)    # [P, d*128] f32 view
                    a_s = ptf.rearrange("p (s e) -> p s e", e=128)[:, :, 96:99]
                    a_d = ndf.rearrange("p (j e) -> p j e", e=128)[:, j, 99:102]

                    et = wpool.tile([P, d * 3], dt.float32, tag="e")
                    nc.vector.tensor_tensor(
                        out=et[:].rearrange("p (s h) -> p s h", h=3),
                        in0=a_s, in1=a_d.unsqueeze(1).to_broadcast([P, d, 3]),
                        op=mybir.AluOpType.add)
                    et2 = wpool.tile([P, d * 3], dt.float32, tag="e2")
                    nc.vector.scalar_tensor_tensor(
                        out=et2[:], in0=et[:], scalar=NEG_SLOPE, in1=et[:],
                        op0=mybir.AluOpType.mult, op1=mybir.AluOpType.max)
                    wf = wpool.tile([P, d * 3], dt.float32, tag="wf")
                    nc.scalar.activation(wf[:], et2[:], mybir.ActivationFunctionType.Exp)
                    wb = wpool.tile([P, d * 3], dt.bfloat16, tag="wb")
                    nc.vector.tensor_copy(out=wb[:], in_=wf[:])
                    st = wpool.tile([P, 3], dt.float32, tag="s")
                    nc.vector.tensor_reduce(
                        out=st[:],
                        in_=wf[:].rearrange("p (s h) -> p h s", h=3),
                        axis=mybir.AxisListType.X, op=mybir.AluOpType.add)
                    # prod = h * w, in place in pt (all operands innermost step-1)
                    h_view = pt[:].rearrange("p (s e) -> p s e", e=E1)[:, :, :F1] \
                                  .rearrange("p s (c h) -> p s c h", h=3)
                    w_view = wb[:].rearrange("p (s h) -> p s h", h=3) \
                                  .unsqueeze(2).to_broadcast([P, d, HID, 3])
                    eng = nc.gpsimd if (mul_gp_mod and t % mul_gp_mod == mul_gp_mod - 1) else nc.vector
                    eng.tensor_tensor(out=h_view, in0=h_view, in1=w_view,
                                      op=mybir.AluOpType.mult)
                    # pairwise tree over slots; level 1 reads pt (stride E1)
                    cur, n = pt, d
                    stride = E1
                    lvl = 0
                    while n > 1:
                        half, odd = n // 2, n % 2
                        dst_t = trp.tile([P, (half + odd) * F1], dt.bfloat16,
                                         tag=f"tr{1 + (lvl % 2)}")
                        slots = cur[:, :n * stride].rearrange(
                            "p (s e) -> p s e", e=stride)[:, :, :F1]
                        nc.vector.tensor_tensor(
                            out=dst_t[:].rearrange("p (s f) -> p s f", f=F1)[:, :half],
                            in0=slots[:, 0:2 * half:2],
                            in1=slots[:, 1:2 * half:2],
                            op=mybir.AluOpType.add)
                        if odd:
                            nc.vector.tensor_copy(
                                out=dst_t[:, half * F1:(half + 1) * F1],
                                in_=cur[:, (n - 1) * stride:(n - 1) * stride + F1])
                        cur, n = dst_t, half + odd
                        stride = F1
                        lvl += 1
                    # normalize + bias + ELU (bf16)
                    sc = wpool.tile([P, 3], dt.float32, tag="sc")
                    nc.vector.tensor_scalar_max(out=sc[:], in0=st[:], scalar1=1e-30)
                    rc = wpool.tile([P, 3], dt.float32, tag="rc")
                    nc.vector.reciprocal(out=rc[:], in_=sc[:])
                    rcb = wpool.tile([P, 3], dt.bfloat16, tag="rcb")
                    nc.vector.tensor_copy(out=rcb[:], in_=rc[:])
                    h1 = hpool.tile([P, F1], dt.bfloat16, tag="h1")
                    nc.vector.tensor_tensor(
                        out=h1[:].rearrange("p (c h) -> p c h", h=3),
                        in0=cur[:, :F1].rearrange("p (c h) -> p c h", h=3),
                        in1=rcb[:].unsqueeze(1).to_broadcast([P, HID, 3]),
                        op=mybir.AluOpType.mult)
                    nc.vector.tensor_add(out=h1[:], in0=h1[:], in1=b1_sb[:])
                    tmin = hpool.tile([P, F1], dt.bfloat16, tag="tmin")
                    nc.vector.tensor_scalar_min(out=tmin[:], in0=h1[:], scalar1=0.0)
                    texp = hpool.tile([P, F1], dt.bfloat16, tag="texp")
                    nc.scalar.activation(texp[:], tmin[:], mybir.ActivationFunctionType.Exp)
                    h1e = hpool.tile([P, F1], dt.bfloat16, tag="h1e")
                    nc.vector.scalar_tensor_tensor(
                        out=h1e[:], in0=h1[:], scalar=0.0, in1=texp[:],
                        op0=mybir.AluOpType.max, op1=mybir.AluOpType.add)
                    nc.vector.tensor_scalar_add(out=h1e[:], in0=h1e[:], scalar1=-1.0)
                    # transpose 192 = 128 + 64
                    tp1 = psB.tile([P, P], dt.bfloat16, tag="tp1")
                    nc.tensor.transpose(tp1[:], h1e[:, :P], ide[:])
                    tp2 = psB.tile([64, P], dt.bfloat16, tag="tp2")
                    nc.tensor.transpose(tp2[:], h1e[:, P:F1], ide[:])
                    hT1 = hpool.tile([P, P], dt.bfloat16, tag="hT1")
                    nc.vector.tensor_copy(out=hT1[:], in_=tp1[:])
                    hT2 = hpool.tile([64, P], dt.bfloat16, tag="hT2")
                    nc.vector.tensor_copy(out=hT2[:], in_=tp2[:])
                    o2 = psA.tile([P, 66], dt.float32, tag="o2")
                    nc.tensor.matmul(o2[:], lhsT=hT1[:], rhs=w2a_sb[:], start=True, stop=False)
                    nc.tensor.matmul(o2[:], lhsT=hT2[:], rhs=w2b_sb[:], start=False, stop=True)
                    r2 = rowp.tile([P, 68], dt.bfloat16, tag="r2")
                    nc.scalar.activation(r2[:, :OUT], o2[:, :OUT],
                                         mybir.ActivationFunctionType.Copy)
                    nc.vector.tensor_copy(out=r2[:, OUT:OUT + 4].bitcast(dt.float32),
                                          in_=o2[:, OUT:OUT + 2])
                    nc.sync.dma_start(out=tab2loc[t * P:(t + 1) * P, :], in_=r2[:])
    nc.compile()
    return nc


def _build_neff2(d_t, mul_gp_mod=0, chunk=CHUNK, pay_bufs=6):
    import concourse.bass as bass
    import concourse.bacc as bacc
    import concourse.mybir as mybir
    import concourse.tile as tile

    dt = mybir.dt
    nc = bacc.Bacc(num_swdge_queues=4)
    CE = int(8 * int(np.sum(d_t)))
    CN = NGRP * (G + 1) * 8
    tab2 = nc.dram_tensor("tab2", [NROW, E2], dt.bfloat16, kind="ExternalInput")
    idxe = nc.dram_tensor("idxe", [P, CE], dt.int16, kind="ExternalInput")
    idxn = nc.dram_tensor("idxn", [P, CN], dt.int16, kind="ExternalInput")
    b2b = nc.dram_tensor("b2b", [P, OUT], dt.float32, kind="ExternalInput")
    out2 = nc.dram_tensor("out2", [T * P, OUT], dt.float32, kind="ExternalOutput")

    FQ = [0]

    def q():
        FQ[0] = (FQ[0] + 1) % 4
        return FQ[0]

    with tile.TileContext(nc) as tc:
        with tc.tile_pool(name="const", bufs=1) as cp, \
             tc.tile_pool(name="gp", bufs=pay_bufs) as gpool, \
             tc.tile_pool(name="np_", bufs=2) as npool, \
             tc.tile_pool(name="wp", bufs=3) as wpool, \
             tc.tile_pool(name="trp", bufs=2) as trp, \
             tc.tile_pool(name="op", bufs=3) as opool:
            ie_sb = cp.tile([P, CE], dt.int16)
            nc.sync.dma_start(out=ie_sb[:], in_=idxe[:, :])
            in_sb = cp.tile([P, CN], dt.int16)
            nc.sync.dma_start(out=in_sb[:], in_=idxn[:, :])
            b2_sb = cp.tile([P, OUT], dt.float32)
            nc.sync.dma_start(out=b2_sb[:], in_=b2b[:, :])

            tab_lo = tab2[BASE:, :]
            off_e = 0
            for g in range(NGRP):
                ndt = npool.tile([P, (G + 1) * E2], dt.bfloat16, tag="nd")
                nc.gpsimd.dma_gather(
                    out_ap=ndt[:].rearrange("p (j e) -> p j e", e=E2),
                    in_ap=tab_lo,
                    idxs_ap=in_sb[:, g * (G + 1) * 8:(g + 1) * (G + 1) * 8],
                    num_idxs=(G + 1) * P, num_idxs_reg=(G + 1) * P,
                    elem_size=E2, single_packet=False, queue_num=q())
                ndf = ndt[:].bitcast(dt.float32)     # [P, (G+1)*64]
                for j in range(G):
                    t = g * G + j
                    d = int(d_t[t])
                    pt = gpool.tile([P, d * E2], dt.bfloat16, tag="pay")
                    nch = (d + chunk - 1) // chunk
                    per = (d + nch - 1) // nch
                    s0 = 0
                    while s0 < d:
                        s1 = min(s0 + per, d)
                        nc.gpsimd.dma_gather(
                            out_ap=pt[:, s0 * E2:s1 * E2].rearrange("p (s e) -> p s e", e=E2),
                            in_ap=tab_lo,
                            idxs_ap=ie_sb[:, off_e + 8 * s0: off_e + 8 * s1],
                            num_idxs=(s1 - s0) * P, num_idxs_reg=(s1 - s0) * P,
                            elem_size=E2, single_packet=False, queue_num=q())
                        s0 = s1
                    off_e += 8 * d
                    ptf = pt[:].bitcast(dt.float32)   # [P, d*64]
                    a_s = ptf.rearrange("p (s e) -> p s e", e=64)[:, :, 32:33]
                    a_d = ndf.rearrange("p (j e) -> p j e", e=64)[:, j, 33:34]
                    et = wpool.tile([P, d], dt.float32, tag="e")
                    nc.vector.tensor_tensor(
                        out=et[:].unsqueeze(2),
                        in0=a_s, in1=a_d.unsqueeze(1).to_broadcast([P, d, 1]),
                        op=mybir.AluOpType.add)
                    et2 = wpool.tile([P, d], dt.float32, tag="e2")
                    nc.vector.scalar_tensor_tensor(
                        out=et2[:], in0=et[:], scalar=NEG_SLOPE, in1=et[:],
                        op0=mybir.AluOpType.mult, op1=mybir.AluOpType.max)
                    wf = wpool.tile([P, d], dt.float32, tag="wf")
                    nc.scalar.activation(wf[:], et2[:], mybir.ActivationFunctionType.Exp)
                    wb = wpool.tile([P, d], dt.bfloat16, tag="wb")
                    nc.vector.tensor_copy(out=wb[:], in_=wf[:])
                    st = wpool.tile([P, 1], dt.float32, tag="s")
                    nc.vector.tensor_reduce(out=st[:], in_=wf[:],
                                            axis=mybir.AxisListType.X,
                                            op=mybir.AluOpType.add)
                    h_view = pt[:].rearrange("p (s e) -> p s e", e=E2)[:, :, :OUT]
                    w_view = wb[:].unsqueeze(2).to_broadcast([P, d, OUT])
                    eng = nc.gpsimd if (mul_gp_mod and t % mul_gp_mod == mul_gp_mod - 1) else nc.vector
                    eng.tensor_tensor(out=h_view, in0=h_view, in1=w_view,
                                      op=mybir.AluOpType.mult)
                    cur, n = pt, d
                    stride = E2
                    lvl = 0
                    while n > 1:
                        half, odd = n // 2, n % 2
                        dst_t = trp.tile([P, (half + odd) * OUT], dt.bfloat16,
                                         tag=f"tr{1 + (lvl % 2)}")
                        nc.vector.tensor_tensor(
                            out=dst_t[:].rearrange("p (s f) -> p s f", f=OUT)[:, :half],
                            in0=cur[:].rearrange("p (s e) -> p s e", e=stride)[:, 0:2 * half:2, :OUT],
                            in1=cur[:].rearrange("p (s e) -> p s e", e=stride)[:, 1:2 * half:2, :OUT],
                            op=mybir.AluOpType.add)
                        if odd:
                            nc.vector.tensor_copy(
                                out=dst_t[:, half * OUT:(half + 1) * OUT],
                                in_=cur[:, (n - 1) * stride:(n - 1) * stride + OUT])
                        cur, n = dst_t, half + odd
                        stride = OUT
                        lvl += 1
                    sc = wpool.tile([P, 1], dt.float32, tag="sc")
                    nc.vector.tensor_scalar_max(out=sc[:], in0=st[:], scalar1=1e-30)
                    rc = wpool.tile([P, 1], dt.float32, tag="rc")
                    nc.vector.reciprocal(out=rc[:], in_=sc[:])
                    ot = opool.tile([P, OUT], dt.float32, tag="ot")
                    nc.vector.tensor_scalar(
                        out=ot[:], in0=cur[:, :OUT], scalar1=rc[:, :1], scalar2=None,
                        op0=mybir.AluOpType.mult)
                    nc.vector.tensor_add(out=ot[:], in0=ot[:], in1=b2_sb[:])
                    nc.sync.dma_start(out=out2[t * P:(t + 1) * P, :], in_=ot[:])
    nc.compile()
    return nc


# ---------------------------------------------------------------- kernel
def kernel(x, edge_index, W1, att_src1, att_dst1, b1, W2, att_src2, att_dst2, b2,
           _emulate=False, _timing=None):
    x = np.asarray(x, np.float32)
    edge_index = np.asarray(edge_index)
    W1 = np.asarray(W1, np.float32)
    att_src1 = np.asarray(att_src1, np.float32)
    att_dst1 = np.asarray(att_dst1, np.float32)
    b1 = np.asarray(b1, np.float32)
    W2 = np.asarray(W2, np.float32)
    att_src2 = np.asarray(att_src2, np.float32)
    att_dst2 = np.asarray(att_dst2, np.float32)
    b2 = np.asarray(b2, np.float32)

    if _emulate:
        return emulate(x, edge_index, W1, att_src1, att_dst1, b1,
                       W2, att_src2, att_dst2, b2)

    from concourse.bass_utils import run_bass_kernel_spmd
    import time as _time

    def _run(nc, maps, trace):
        try:
            return run_bass_kernel_spmd(nc, maps, core_ids=list(range(NCORE)), trace=trace)
        except Exception:
            _time.sleep(25)
            return run_bass_kernel_spmd(nc, maps, core_ids=list(range(NCORE)), trace=False)

    pre = preprocess(edge_index)
    hw = host_weights(x, W1, att_src1, att_dst1, b1, W2, att_src2, att_dst2, b2)
    d_t, nid = pre["d_t"], pre["nid"]

    trace = _timing is not None

    # ---- NEFF1
    nc1 = _build_neff1(d_t)
    maps1 = [dict(xT=hw["xT"], W1e=hw["W1e"], W2e1=hw["W2e1"], W2e2=hw["W2e2"],
                  b1i=hw["b1i"], sent1=hw["sent1"],
                  idxe=pre["idxe"][c], idxn=pre["idxn"][c]) for c in range(NCORE)]
    res1 = _run(nc1, maps1, trace)

    # host: assemble global table2
    tab2 = np.zeros((NROW, E2), bf16)
    tab2[SENT].view(np.float32)[32] = -1e30
    for c in range(NCORE):
        loc = res1.results[c]["tab2loc"]           # [T*P, 68] bf16
        nn = nid[c].reshape(-1)                    # [T*P]
        valid = nn != SENT
        tab2[nn[valid], :68] = loc[valid]

    # ---- NEFF2
    nc2 = _build_neff2(d_t)
    maps2 = [dict(tab2=tab2, idxe=pre["idxe"][c], idxn=pre["idxn"][c],
                  b2b=hw["b2b"]) for c in range(NCORE)]
    res2 = _run(nc2, maps2, trace)

    out = np.zeros((N, OUT), np.float32)
    for c in range(NCORE):
        o = res2.results[c]["out2"]
        nn = nid[c].reshape(-1)
        valid = nn != SENT
        out[nn[valid]] = o[valid]

    if _timing is not None:
        _timing["neff1_ns"] = res1.exec_time_ns
        _timing["neff2_ns"] = res2.exec_time_ns
    return out
